# revision 1
# baseline (speedup 1.0000x reference)
"""Multi-head attention (projections + causal/padded softmax attention + output
projection + residual + LayerNorm) as a Bass/Tile kernel on 8 Trainium2 cores.

Sharding: tensor-parallel over heads within each batch. Core c handles batch
b = c // 4 and heads [4*(c%4), 4*(c%4)+4). Each core projects Q/K/V for its
4 heads over the full sequence, runs causal attention in a transposed layout
(scoresT[key, row]), and produces ctxT[dh, row]. Two 8-way AllToAlls (one per
head-pair, so the first overlaps the second pair's attention) redistribute
ctxT so core c ends with the full context dims for its 512-row quarter, on
which it runs the output projection, residual add and LayerNorm.

Layout trick: all matmul operands are pre-transposed/pre-cast on the host
(numpy) so every DMA is contiguous: qT/kT/vT = x^T as bf16, WqT/WkT/WvT/WoT =
W^T as bf16. The PE contracts over partitions, so the contraction dim (d_model
or d_head) always sits on the partition axis.

Softmax: scores are bounded (|s| ~ 5) so exp is computed without max
subtraction; exp(scale*s + pad_bias) runs on the scalar engine with the
padding mask folded into the per-key bias. The causal boundary is enforced by
zeroing probs with gpsimd.affine_select. The denominator is obtained by
augmenting V with a ones column (row 64 of ctxT psum = sum of probs); the
divide happens as broadcast+reciprocal+multiply on 64 partitions.

PSUM budget (8 banks): pj=3 (projection accumulators, reused as two Wo
halves in P3), sc=3 (score chunks, both heads round-robin), ctx=2.
"""

import math
from contextlib import ExitStack

import numpy as np
import ml_dtypes

import concourse.bass as bass
import concourse.mybir as mybir
import concourse.tile as tile
from concourse import bacc
from concourse.bass import ds
from concourse.bass_utils import run_bass_kernel_spmd

BF16 = mybir.dt.bfloat16
F32 = mybir.dt.float32

NEG_INF = -1e9
LN_EPS = 1e-6


class Cfg:
    def __init__(self, B=2, S=2048, D=1024, H=16, dh=64, kmax=None):
        self.B, self.S, self.D, self.H, self.dh = B, S, D, H, dh
        # kmax: max(sen_len) — keys beyond are fully masked, so K/V
        # projection and the attention key loop stop at this bound.
        self.kmax = S if kmax is None else min(int(kmax), S)
        self.NC = 8                      # cores
        self.G = 4                       # cores per batch group
        self.HPC = H // self.G           # heads per core
        self.PAIRS = self.HPC // 2       # head pairs per core
        self.D4 = self.HPC * dh          # per-core projection width
        self.RQ = S // self.G            # rows per core in Wo/LN phase
        self.NR = 4                      # attention row ranges
        self.RNG = S // self.NR          # rows per range (== RQ)
        self.DC = D // 128               # contraction chunks
        self.KCH = S // 128              # key chunks
        self.NS = max(1, S // 512)       # projection n-slices
        self.NSW = S // self.NS          # cols per n-slice
        self.WON = max(1, D // 512)      # Wo n-slices
        self.WONW = D // self.WON
        self.D4C = self.D4 // 128        # 128-chunks in per-core ctx width
        self.KB_MAX = -(-self.kmax // 128)          # key chunks actually used
        self.NS_K = -(-(self.KB_MAX * 128) // self.NSW)  # K-proj n-slices
        assert self.RQ == self.RNG
        assert self.PAIRS >= 1 and self.HPC % 2 == 0


def build_program(cfg: Cfg, debug_taps: bool = False):
    """Build the (SPMD-identical) Bass program."""
    nc = bacc.Bacc("TRN2", target_bir_lowering=False, debug=False,
                   num_devices=cfg.NC)

    S, D, dh = cfg.S, cfg.D, cfg.dh
    D4, RQ, RNG = cfg.D4, cfg.RQ, cfg.RNG

    qT = nc.dram_tensor("qT", [D, S], BF16, kind="ExternalInput").ap()
    kT = nc.dram_tensor("kT", [D, S], BF16, kind="ExternalInput").ap()
    vT = nc.dram_tensor("vT", [D, S], BF16, kind="ExternalInput").ap()
    wqT = nc.dram_tensor("wqT", [D, D4], BF16, kind="ExternalInput").ap()
    wkT = nc.dram_tensor("wkT", [D, D4], BF16, kind="ExternalInput").ap()
    wvT = nc.dram_tensor("wvT", [D, D4], BF16, kind="ExternalInput").ap()
    woT = nc.dram_tensor("woT", [D, D], BF16, kind="ExternalInput").ap()
    resid = nc.dram_tensor("resid", [RQ, D], F32, kind="ExternalInput").ap()
    pad_bias = nc.dram_tensor("pad_bias", [cfg.KCH, 128], F32,
                              kind="ExternalInput").ap()
    gamma = nc.dram_tensor("gamma", [1, D], F32, kind="ExternalInput").ap()
    beta = nc.dram_tensor("beta", [1, D], F32, kind="ExternalInput").ap()
    out_shard = nc.dram_tensor("out_shard", [RQ, D], F32,
                               kind="ExternalOutput").ap()
    if debug_taps:
        dbg_khT = nc.dram_tensor("dbg_khT", [128, cfg.PAIRS, S], BF16,
                                 kind="ExternalOutput").ap()
        dbg_qhT = nc.dram_tensor("dbg_qhT", [128, cfg.PAIRS, S], BF16,
                                 kind="ExternalOutput").ap()
        dbg_vh = nc.dram_tensor("dbg_vh", [128, cfg.KCH,
                                           cfg.HPC * (dh + 1)], BF16,
                                kind="ExternalOutput").ap()
        dbg_a2ain = nc.dram_tensor("dbg_a2ain", [cfg.NC, 128, RQ], BF16,
                                   kind="ExternalOutput").ap()
        dbg_a2aout = nc.dram_tensor("dbg_a2aout", [cfg.NC, 128, RQ], BF16,
                                    kind="ExternalOutput").ap()
        dbg_rbc = nc.dram_tensor("dbg_rbc", [128, RNG], F32,
                                 kind="ExternalOutput").ap()
        dbg_sc = nc.dram_tensor("dbg_sc", [128, RNG], F32,
                                kind="ExternalOutput").ap()
        dbg_probs = nc.dram_tensor("dbg_probs", [128, RNG], BF16,
                                   kind="ExternalOutput").ap()
        dbg_probs6 = nc.dram_tensor("dbg_probs6", [128, RNG], BF16,
                                    kind="ExternalOutput").ap()
        dbg_ctx = nc.dram_tensor("dbg_ctx", [dh + 1, RNG], F32,
                                 kind="ExternalOutput").ap()
        dbg_dbc = nc.dram_tensor("dbg_dbc", [128, RNG], F32,
                                 kind="ExternalOutput").ap()

    with tile.TileContext(nc) as tc, ExitStack() as ctx:
        consts = ctx.enter_context(tc.tile_pool(name="consts", bufs=1))
        xin = ctx.enter_context(tc.tile_pool(name="xin", bufs=2))
        proj = ctx.enter_context(tc.tile_pool(name="proj", bufs=1))
        att = ctx.enter_context(tc.tile_pool(name="att", bufs=4))
        small = ctx.enter_context(tc.tile_pool(name="small", bufs=4))
        lnp = ctx.enter_context(tc.tile_pool(name="lnp", bufs=2))
        ctxf = ctx.enter_context(tc.tile_pool(name="ctxf", bufs=1))
        dram = ctx.enter_context(
            tc.tile_pool(name="dram", bufs=1, space="DRAM"))
        psum = ctx.enter_context(
            tc.tile_pool(name="psum", bufs=1, space="PSUM"))

        # ---- prologue: constants (wo/gamma/beta deferred to P3) ------------
        wq_sb = consts.tile([128, cfg.DC, D4], BF16)
        wk_sb = consts.tile([128, cfg.DC, D4], BF16)
        wv_sb = consts.tile([128, cfg.DC, D4], BF16)
        for w_sb, w_dram in ((wk_sb, wkT), (wv_sb, wvT), (wq_sb, wqT)):
            nc.sync.dma_start(
                out=w_sb, in_=w_dram.rearrange("(c p) o -> p c o", p=128))

        pb_sb = consts.tile([128, cfg.KCH], F32)
        nc.sync.dma_start(out=pb_sb, in_=pad_bias.rearrange("c p -> p c"))

        # batch predicates: core c belongs to batch c // G. All A2A
        # staging/output DMAs use static addresses predicated on these, so
        # Tile tracks the dependencies exactly (dynamic register offsets
        # proved unreliable to order against the collective on HW).
        pid = nc.gpsimd.partition_id()
        blk4 = nc.gpsimd.scalar_reg_alu(mybir.AluOpType.bitwise_and, pid,
                                        cfg.G)
        blk = blk4

        a2a_in = dram.tile([cfg.NC, cfg.PAIRS, 128, RQ], BF16,
                           name="a2a_in")
        a2a_out = dram.tile([cfg.NC, cfg.PAIRS, 128, RQ], BF16,
                            name="a2a_out")

        # ---- P1: projections (K, V first so attention can start early) ----
        qhT_sb = proj.tile([128, cfg.PAIRS, S], BF16)
        khT_sb = proj.tile([128, cfg.PAIRS, S], BF16)
        vh_sb = proj.tile([128, cfg.KCH, cfg.HPC * (dh + 1)], BF16)

        def qk_proj(x_dram, w_sb, out_sb, ns_count=None):
            for ns in range(ns_count if ns_count is not None else cfg.NS):
                x_ns = xin.tile([128, cfg.DC, cfg.NSW], BF16, tag="x_ns",
                                name="x_ns")
                nc.sync.dma_start(
                    out=x_ns, in_=x_dram.rearrange("(c p) s -> p c s", p=128)
                    [:, :, ns * cfg.NSW:(ns + 1) * cfg.NSW])
                for pair in range(cfg.PAIRS):
                    ps = psum.tile([128, cfg.NSW], F32, tag="pj", bufs=3,
                                   name="ps_pj")
                    for dc in range(cfg.DC):
                        nc.tensor.matmul(
                            ps, w_sb[:, dc, pair * 128:(pair + 1) * 128],
                            x_ns[:, dc, :],
                            start=dc == 0, stop=dc == cfg.DC - 1)
                    nc.vector.tensor_copy(
                        out=out_sb[:, pair, ns * cfg.NSW:(ns + 1) * cfg.NSW],
                        in_=ps)

        qk_proj(kT, wk_sb, khT_sb, ns_count=cfg.NS_K)

        for kb in range(cfg.KB_MAX):
            v_kb = xin.tile([128, cfg.DC, 128], BF16, tag="v_kb")
            nc.sync.dma_start(
                out=v_kb, in_=vT.rearrange("(c p) s -> p c s", p=128)
                [:, :, kb * 128:(kb + 1) * 128])
            psv = psum.tile([128, D4], F32, tag="pj", bufs=3, name="ps_v")
            for dc in range(cfg.DC):
                nc.tensor.matmul(psv, v_kb[:, dc, :], wv_sb[:, dc, :],
                                 start=dc == 0, stop=dc == cfg.DC - 1)
            nc.vector.tensor_copy(
                out=vh_sb[:, kb, :].rearrange("p (h e) -> p h e", e=dh + 1)
                [:, :, 0:dh],
                in_=psv.rearrange("p (h e) -> p h e", e=dh))
            nc.vector.memset(
                vh_sb[:, kb, :].rearrange("p (h e) -> p h e", e=dh + 1)
                [:, :, dh:dh + 1], 1.0)

        qk_proj(qT, wq_sb, qhT_sb)

        # ---- P2: attention; per-pair A2A issued as soon as pair finishes ---
        ccb = {}
        for pair in range(cfg.PAIRS):
            for r in range(cfg.NR):
                nch = min(((r + 1) * RNG) // 128, cfg.KB_MAX)
                ctx_ps = [psum.tile([dh + 1, RNG], F32, tag=f"ctx{h2}",
                                    bufs=1, name=f"ctx_ps{h2}")
                          for h2 in range(2)]
                for kb in range(nch):
                    # causal column truncation: rows r*RNG+f with f < f0
                    # (= kb*128 - r*RNG) are entirely below the diagonal.
                    f0 = max(0, kb * 128 - r * RNG)
                    w = RNG - f0
                    sc = [psum.tile([128, RNG], F32, tag="sc", bufs=3,
                                    name=f"sc{h2}") for h2 in range(2)]
                    probs = [att.tile([128, RNG], BF16, tag=f"pr{h2}",
                                      name=f"probs{h2}") for h2 in range(2)]
                    for h2 in range(2):
                        lo, hi = 64 * h2, 64 * h2 + 64
                        nc.tensor.matmul(
                            sc[h2][:, 0:w],
                            khT_sb[lo:hi, pair, kb * 128:(kb + 1) * 128],
                            qhT_sb[lo:hi, pair,
                                   r * RNG + f0:(r + 1) * RNG],
                            start=True, stop=True)
                        nc.scalar.activation(
                            out=probs[h2][:, f0:], in_=sc[h2][:, 0:w],
                            func=mybir.ActivationFunctionType.Exp,
                            bias=pb_sb[:, kb:kb + 1],
                            scale=1.0 / math.sqrt(dh))
                        if f0 > 0 or kb * 128 == r * RNG:
                            # partial band: keep f - f0 >= p
                            nc.gpsimd.affine_select(
                                out=probs[h2][:, f0:f0 + 128],
                                in_=probs[h2][:, f0:f0 + 128],
                                pattern=[[1, 128]],
                                base=0,
                                channel_multiplier=-1,
                                compare_op=mybir.AluOpType.is_ge,
                                fill=0.0)
                        h = 2 * pair + h2
                        nc.tensor.matmul(
                            ctx_ps[h2][:, f0:],
                            vh_sb[:, kb, h * (dh + 1):(h + 1) * (dh + 1)],
                            probs[h2][:, f0:],
                            start=kb == 0, stop=kb == nch - 1)
                # epilogue: divide by denominator (row dh of ctx psum).
                # Pool can't read PSUM, so bounce the denom row via SBUF,
                # broadcast to 64 partitions, then reciprocal+mul run wide.
                stage = att.tile([128, RNG], BF16, tag="stage")
                for h2 in range(2):
                    den = small.tile([1, RNG], F32, tag="den", name="den")
                    nc.vector.tensor_copy(out=den,
                                          in_=ctx_ps[h2][dh:dh + 1, :])
                    dbc = small.tile([64, RNG], F32, tag="dbc", name="dbc")
                    nc.gpsimd.partition_broadcast(dbc, den)
                    rbc = small.tile([64, RNG], F32, tag="rbc", name="rbc")
                    nc.vector.reciprocal(rbc, dbc)
                    nc.vector.tensor_mul(
                        stage[64 * h2:64 * h2 + 64, :],
                        ctx_ps[h2][0:dh, :], rbc)
                nc.gpsimd.dma_start(
                    out=a2a_in[ds(blk + r, 1), pair, :, :], in_=stage)
        # hard barrier        # hard barrier: every staging DMA must have fully landed before the
        # collective reads a2a_in (observed stale-read corruption of the
        # last-staged shards without it)
        tc.strict_bb_all_engine_barrier()
        nc.gpsimd.collective_compute(
            "AllToAll", mybir.AluOpType.bypass,
            replica_groups=[list(range(cfg.NC))],
            ins=[a2a_in[:]], outs=[a2a_out[:]])
        tc.strict_bb_all_engine_barrier()
        for pair in range(cfg.PAIRS):
            for l in range(cfg.G):
                t_ccb = ctxf.tile([128, RQ], BF16, name=f"ccb_{pair}_{l}",
                                  tag=f"ccb_{pair}_{l}")
                nc.gpsimd.dma_start(
                    out=t_ccb, in_=a2a_out[ds(blk + l, 1), pair, :, :])
                ccb[(pair, l)] = t_ccb

        if debug_taps:
            nc.sync.dma_start(out=dbg_khT, in_=khT_sb)
            nc.sync.dma_start(out=dbg_qhT, in_=qhT_sb)
            nc.sync.dma_start(out=dbg_vh, in_=vh_sb)
            nc.gpsimd.dma_start(out=dbg_a2ain, in_=a2a_in[:, 0, :, :])
            nc.gpsimd.dma_start(out=dbg_a2aout, in_=a2a_out[:, 0, :, :])

        # ---- P3: Wo + residual + LayerNorm ---------------------------------
        wo_sb = consts.tile([128, cfg.DC, D], BF16)
        nc.sync.dma_start(out=wo_sb,
                          in_=woT.rearrange("(c p) o -> p c o", p=128))
        g_row = consts.tile([1, D], F32)
        b_row = consts.tile([1, D], F32)
        nc.sync.dma_start(out=g_row, in_=gamma)
        nc.sync.dma_start(out=b_row, in_=beta)
        gamma_bc = consts.tile([128, D], F32)
        beta_bc = consts.tile([128, D], F32)
        nc.gpsimd.partition_broadcast(gamma_bc, g_row)
        nc.gpsimd.partition_broadcast(beta_bc, b_row)
        eps_sb = consts.tile([128, 1], F32)
        nc.vector.memset(eps_sb, LN_EPS)

        n_jc = cfg.G * cfg.D4C  # total 128-chunks of context width D
        for t in range(RQ // 128):
            pso = [psum.tile([128, cfg.WONW], F32, tag="pj", bufs=3,
                             name=f"pso{nsl}") for nsl in range(cfg.WON)]
            for jc in range(n_jc):
                # global dh-chunk jc lives in a2a buffer of pair p, block l
                l, p = divmod(jc, cfg.PAIRS)
                cc = ccb[(p, l)][:, t * 128:(t + 1) * 128]
                for nsl in range(cfg.WON):
                    nc.tensor.matmul(
                        pso[nsl], cc,
                        wo_sb[:, jc, nsl * cfg.WONW:(nsl + 1) * cfg.WONW],
                        start=jc == 0, stop=jc == n_jc - 1)
            res = lnp.tile([128, D], F32, tag="res")
            nc.sync.dma_start(out=res, in_=resid[t * 128:(t + 1) * 128, :])
            x = lnp.tile([128, D], F32, tag="x")
            for nsl in range(cfg.WON):
                sl = slice(nsl * cfg.WONW, (nsl + 1) * cfg.WONW)
                nc.vector.tensor_add(x[:, sl], pso[nsl], res[:, sl])
            fmax = math.gcd(nc.vector.BN_STATS_FMAX, D)
            nsub = D // fmax
            stats = lnp.tile([128, nsub, nc.vector.BN_STATS_DIM], F32,
                             tag="stats")
            for sg in range(nsub):
                nc.vector.bn_stats(
                    out=stats[:, sg, :],
                    in_=x.rearrange("p (a b) -> p a b", a=nsub)[:, sg, :])
            mv = lnp.tile([128, nc.vector.BN_AGGR_DIM], F32, tag="mv")
            nc.vector.bn_aggr(out=mv, in_=stats)
            sd = lnp.tile([128, 1], F32, tag="sd")
            nc.scalar.activation(out=sd, in_=mv[:, 1:2],
                                 func=mybir.ActivationFunctionType.Sqrt,
                                 bias=eps_sb, scale=1.0)
            rstd = lnp.tile([128, 1], F32, tag="rstd")
            nc.vector.reciprocal(rstd, sd)
            y = lnp.tile([128, D], F32, tag="y")
            nc.vector.tensor_scalar(
                out=y, in0=x, scalar1=mv[:, 0:1], scalar2=rstd,
                op0=mybir.AluOpType.subtract, op1=mybir.AluOpType.mult)
            yg = lnp.tile([128, D], F32, tag="yg")
            nc.vector.tensor_mul(yg, y, gamma_bc)
            out_sb = lnp.tile([128, D], F32, tag="out_sb")
            nc.vector.tensor_add(out_sb, yg, beta_bc)
            nc.sync.dma_start(out=out_shard[t * 128:(t + 1) * 128, :],
                              in_=out_sb)

    nc.compile()
    return nc


def make_in_maps(cfg: Cfg, q, k, v, Wq, Wk, Wv, Wo, gamma, beta, sen_len):
    """Host-side sharding: slice/transpose/cast per core."""
    bf = ml_dtypes.bfloat16
    in_maps = []
    woT_full = np.ascontiguousarray(Wo.T.astype(bf))
    pos = np.arange(cfg.S)
    per_batch = {}
    for b in range(cfg.B):
        per_batch[b] = (
            np.ascontiguousarray(q[b].T.astype(bf)),
            np.ascontiguousarray(k[b].T.astype(bf)),
            np.ascontiguousarray(v[b].T.astype(bf)),
            np.where(pos < int(sen_len[b]), 0.0, NEG_INF).astype(np.float32),
        )
    for c in range(cfg.NC):
        b = c // cfg.G
        l = c % cfg.G
        hs = slice(l * cfg.D4, (l + 1) * cfg.D4)
        rows = slice(l * cfg.RQ, (l + 1) * cfg.RQ)
        qTb, kTb, vTb, pb = per_batch[b]
        in_maps.append({
            "qT": qTb, "kT": kTb, "vT": vTb,
            "wqT": np.ascontiguousarray(Wq[hs, :].T.astype(bf)),
            "wkT": np.ascontiguousarray(Wk[hs, :].T.astype(bf)),
            "wvT": np.ascontiguousarray(Wv[hs, :].T.astype(bf)),
            "woT": woT_full,
            "resid": np.ascontiguousarray(q[b, rows, :]).astype(np.float32),
            "pad_bias": pb.reshape(cfg.KCH, 128),
            "gamma": gamma.reshape(1, cfg.D).astype(np.float32),
            "beta": beta.reshape(1, cfg.D).astype(np.float32),
        })
    return in_maps


def assemble_output(cfg: Cfg, results):
    out = np.empty((cfg.B, cfg.S, cfg.D), np.float32)
    for c in range(cfg.NC):
        b, l = c // cfg.G, c % cfg.G
        out[b, l * cfg.RQ:(l + 1) * cfg.RQ, :] = results[c]["out_shard"]
    return out


_PROGRAM_CACHE = {}


def _get_program(cfg: Cfg):
    key = (cfg.B, cfg.S, cfg.D, cfg.H, cfg.dh, cfg.KB_MAX)
    if key not in _PROGRAM_CACHE:
        _PROGRAM_CACHE[key] = build_program(cfg)
    return _PROGRAM_CACHE[key]


def run(cfg: Cfg, inputs: dict, trace: bool = False):
    nc = _get_program(cfg)
    in_maps = make_in_maps(cfg, **inputs)
    res = run_bass_kernel_spmd(nc, in_maps, core_ids=list(range(cfg.NC)),
                               trace=trace)
    return assemble_output(cfg, res.results), res


def kernel(**inputs) -> np.ndarray:
    kmax = int(np.max(inputs["sen_len"]))
    cfg = Cfg(B=2, S=2048, D=1024, H=16, dh=64, kmax=kmax)
    out, _ = run(cfg, inputs)
    return out



# revision 3
# speedup vs baseline: 1.3290x; 1.3290x over previous
"""Multi-head attention (projections + causal/padded softmax attention + output
projection + residual + LayerNorm) as a Bass/Tile kernel on 8 Trainium2 cores.

Sharding: tensor-parallel over heads within each batch. Core c handles batch
g = c // 4 and heads [4*(c%4), 4*(c%4)+4). Each core projects Q/K/V for its
4 heads over the full sequence, runs causal attention in a transposed layout
(scoresT[key, row]), and produces ctxT[dh, row]. One 8-way AllToAll per
head-pair redistributes ctxT with a fully STATIC slot map: slot j carries rows
[j*256, (j+1)*256) of the sender's batch, so core j ends up owning that row
range of BOTH batches (cores 0-3 receive batch-0 contributions from cores 0-3
and batch-1 contributions from cores 4-7 in distinct sender slots). No runtime
core-id addressing, no barriers: Tile orders staging DMAs before each
collective and the pair-0 collective overlaps pair-1's attention.

Layout trick: all matmul operands are pre-transposed/pre-cast on the host
(numpy) so every DMA is contiguous: qT/kT/vT = x^T as bf16, WqT/WkT/WvT/WoT =
W^T as bf16. The PE contracts over partitions, so the contraction dim (d_model
or d_head) always sits on the partition axis.

Softmax: scores are bounded (|s| ~ 5) so exp is computed without max
subtraction; exp(scale*s + pad_bias) runs on the scalar engine with the
padding mask folded into the per-key bias. The causal boundary is enforced by
zeroing probs with gpsimd.affine_select. The denominator is obtained by
augmenting V with a ones column (row 64 of ctxT psum = sum of probs).

Attention is software-pipelined for the PE p-state ramp: the ctx matmul of
chunk kb is emitted between the score matmuls of chunk kb+1 so the tensor
engine never waits on the scalar-engine exp. The softmax epilogue first copies
the ctx psum to SBUF (freeing the bank), then does a cheap [1,R] reciprocal,
partition-broadcast and multiply off the critical path.

PSUM budget (8 banks): pj=2 + sc=3 + ctx=2 = 7 (pj/sc shapes are reused for
the Wo accumulators in P3).
"""

import math
from contextlib import ExitStack

import numpy as np
import ml_dtypes

import concourse.bass as bass
import concourse.mybir as mybir
import concourse.tile as tile
from concourse import bacc
from concourse.bass_utils import run_bass_kernel_spmd

BF16 = mybir.dt.bfloat16
F32 = mybir.dt.float32

NEG_INF = -1e9
LN_EPS = 1e-6


class Cfg:
    def __init__(self, B=2, S=2048, D=1024, H=16, dh=64, kmax=None):
        self.B, self.S, self.D, self.H, self.dh = B, S, D, H, dh
        # kmax: max(sen_len) — keys beyond are fully masked, so K/V
        # projection and the attention key loop stop at this bound.
        self.kmax = S if kmax is None else min(int(kmax), S)
        self.NC = 8                      # cores
        self.G = 4                       # cores per batch group
        self.HPC = H // self.G           # heads per core
        self.PAIRS = self.HPC // 2       # head pairs per core
        self.D4 = self.HPC * dh          # per-core projection width
        self.RQ = S // self.G            # rows per core in Wo/LN phase
        self.NR = 4                      # attention row ranges
        self.RNG = S // self.NR          # rows per range (== RQ)
        self.RSL = S // self.NC          # rows per A2A slot (256)
        self.DC = D // 128               # contraction chunks
        self.KCH = S // 128              # key chunks
        self.NS = max(1, S // 512)       # projection n-slices
        self.NSW = S // self.NS          # cols per n-slice
        self.WON = max(1, D // 512)      # Wo n-slices
        self.WONW = D // self.WON
        self.D4C = self.D4 // 128        # 128-chunks in per-core ctx width
        self.KB_MAX = -(-self.kmax // 128)          # key chunks actually used
        self.NS_K = -(-(self.KB_MAX * 128) // self.NSW)  # K-proj n-slices
        assert self.RQ == self.RNG
        assert self.PAIRS >= 1 and self.HPC % 2 == 0


def build_program(cfg: Cfg):
    """Build the (SPMD-identical) Bass program."""
    nc = bacc.Bacc("TRN2", target_bir_lowering=False, debug=False,
                   num_devices=cfg.NC)

    S, D, dh = cfg.S, cfg.D, cfg.dh
    D4, RQ, RNG, RSL = cfg.D4, cfg.RQ, cfg.RNG, cfg.RSL

    qT = nc.dram_tensor("qT", [D, S], BF16, kind="ExternalInput").ap()
    kT = nc.dram_tensor("kT", [D, S], BF16, kind="ExternalInput").ap()
    vT = nc.dram_tensor("vT", [D, S], BF16, kind="ExternalInput").ap()
    wqT = nc.dram_tensor("wqT", [D, D4], BF16, kind="ExternalInput").ap()
    wkT = nc.dram_tensor("wkT", [D, D4], BF16, kind="ExternalInput").ap()
    wvT = nc.dram_tensor("wvT", [D, D4], BF16, kind="ExternalInput").ap()
    woT = nc.dram_tensor("woT", [D, D], BF16, kind="ExternalInput").ap()
    resid = nc.dram_tensor("resid", [RQ, D], F32, kind="ExternalInput").ap()
    pad_bias = nc.dram_tensor("pad_bias", [cfg.KCH, 128], F32,
                              kind="ExternalInput").ap()
    gamma = nc.dram_tensor("gamma", [1, D], F32, kind="ExternalInput").ap()
    beta = nc.dram_tensor("beta", [1, D], F32, kind="ExternalInput").ap()
    out_shard = nc.dram_tensor("out_shard", [RQ, D], F32,
                               kind="ExternalOutput").ap()

    with tile.TileContext(nc) as tc, ExitStack() as ctx:
        consts = ctx.enter_context(tc.tile_pool(name="consts", bufs=1))
        xin = ctx.enter_context(tc.tile_pool(name="xin", bufs=2))
        proj = ctx.enter_context(tc.tile_pool(name="proj", bufs=1))
        att = ctx.enter_context(tc.tile_pool(name="att", bufs=2))
        small = ctx.enter_context(tc.tile_pool(name="small", bufs=2))
        lnp = ctx.enter_context(tc.tile_pool(name="lnp", bufs=2))
        ctxf = ctx.enter_context(tc.tile_pool(name="ctxf", bufs=1))
        dram = ctx.enter_context(
            tc.tile_pool(name="dram", bufs=1, space="DRAM"))
        psum = ctx.enter_context(
            tc.tile_pool(name="psum", bufs=1, space="PSUM"))

        # ---- prologue: all constants (incl. P3's, so P3 never waits) -------
        wq_sb = consts.tile([128, cfg.DC, D4], BF16)
        wk_sb = consts.tile([128, cfg.DC, D4], BF16)
        wv_sb = consts.tile([128, cfg.DC, D4], BF16)
        for w_sb, w_dram in ((wk_sb, wkT), (wv_sb, wvT), (wq_sb, wqT)):
            nc.sync.dma_start(
                out=w_sb, in_=w_dram.rearrange("(c p) o -> p c o", p=128))

        pb_sb = consts.tile([128, cfg.KCH], F32)
        nc.sync.dma_start(out=pb_sb, in_=pad_bias.rearrange("c p -> p c"))

        wo_sb = consts.tile([128, cfg.DC, D], BF16)
        nc.sync.dma_start(out=wo_sb,
                          in_=woT.rearrange("(c p) o -> p c o", p=128))
        g_row = consts.tile([1, D], F32)
        b_row = consts.tile([1, D], F32)
        nc.sync.dma_start(out=g_row, in_=gamma)
        nc.sync.dma_start(out=b_row, in_=beta)
        gamma_bc = consts.tile([128, D], F32)
        beta_bc = consts.tile([128, D], F32)
        nc.gpsimd.partition_broadcast(gamma_bc, g_row)
        nc.gpsimd.partition_broadcast(beta_bc, b_row)
        eps_sb = consts.tile([128, 1], F32)
        nc.vector.memset(eps_sb, LN_EPS)
        res_sb = consts.tile([128, cfg.G, D], F32)
        nc.sync.dma_start(
            out=res_sb, in_=resid.rearrange("(t p) d -> p t d", p=128))

        # A2A buffers: one per head-pair; slot j = rows [j*RSL,(j+1)*RSL).
        a2a_in = [dram.tile([cfg.NC, 128, RSL], BF16, name=f"a2a_in{p}",
                            tag=f"a2a_in{p}") for p in range(cfg.PAIRS)]
        a2a_out = [dram.tile([cfg.NC, 128, RSL], BF16, name=f"a2a_out{p}",
                             tag=f"a2a_out{p}") for p in range(cfg.PAIRS)]

        # ---- P1: projections (K, V first so attention can start early) ----
        qhT_sb = proj.tile([128, cfg.PAIRS, S], BF16)
        khT_sb = proj.tile([128, cfg.PAIRS, S], BF16)
        vh_sb = proj.tile([128, cfg.KCH, cfg.HPC * (dh + 1)], BF16)

        def qk_proj(x_dram, w_sb, out_sb, ns_count=None):
            for ns in range(ns_count if ns_count is not None else cfg.NS):
                x_ns = xin.tile([128, cfg.DC, cfg.NSW], BF16, tag="x_ns",
                                name="x_ns")
                nc.sync.dma_start(
                    out=x_ns, in_=x_dram.rearrange("(c p) s -> p c s", p=128)
                    [:, :, ns * cfg.NSW:(ns + 1) * cfg.NSW])
                for pair in range(cfg.PAIRS):
                    ps = psum.tile([128, cfg.NSW], F32, tag="pj", bufs=2,
                                   name="ps_pj")
                    for dc in range(cfg.DC):
                        nc.tensor.matmul(
                            ps, w_sb[:, dc, pair * 128:(pair + 1) * 128],
                            x_ns[:, dc, :],
                            start=dc == 0, stop=dc == cfg.DC - 1)
                    nc.vector.tensor_copy(
                        out=out_sb[:, pair, ns * cfg.NSW:(ns + 1) * cfg.NSW],
                        in_=ps)

        qk_proj(kT, wk_sb, khT_sb, ns_count=cfg.NS_K)

        for kb in range(cfg.KB_MAX):
            v_kb = xin.tile([128, cfg.DC, 128], BF16, tag="v_kb")
            nc.sync.dma_start(
                out=v_kb, in_=vT.rearrange("(c p) s -> p c s", p=128)
                [:, :, kb * 128:(kb + 1) * 128])
            psv = psum.tile([128, D4], F32, tag="pj", bufs=2, name="ps_v")
            for dc in range(cfg.DC):
                nc.tensor.matmul(psv, v_kb[:, dc, :], wv_sb[:, dc, :],
                                 start=dc == 0, stop=dc == cfg.DC - 1)
            nc.vector.tensor_copy(
                out=vh_sb[:, kb, :].rearrange("p (h e) -> p h e", e=dh + 1)
                [:, :, 0:dh],
                in_=psv.rearrange("p (h e) -> p h e", e=dh))
            nc.vector.memset(
                vh_sb[:, kb, :].rearrange("p (h e) -> p h e", e=dh + 1)
                [:, :, dh:dh + 1], 1.0)

        qk_proj(qT, wq_sb, qhT_sb)

        # ---- P2: attention; per-pair A2A overlaps the next pair -----------
        def score_mm(pair, r, kb, h2, sc, probs):
            """Score matmul + exp (+ causal select) for one chunk/head."""
            f0 = max(0, kb * 128 - r * RNG)
            w = RNG - f0
            lo, hi = 64 * h2, 64 * h2 + 64
            nc.tensor.matmul(
                sc[:, 0:w],
                khT_sb[lo:hi, pair, kb * 128:(kb + 1) * 128],
                qhT_sb[lo:hi, pair, r * RNG + f0:(r + 1) * RNG],
                start=True, stop=True)
            nc.scalar.activation(
                out=probs[:, f0:], in_=sc[:, 0:w],
                func=mybir.ActivationFunctionType.Exp,
                bias=pb_sb[:, kb:kb + 1],
                scale=1.0 / math.sqrt(dh))
            if f0 > 0 or kb * 128 == r * RNG:
                # partial band: keep f - f0 >= p
                nc.gpsimd.affine_select(
                    out=probs[:, f0:f0 + 128],
                    in_=probs[:, f0:f0 + 128],
                    pattern=[[1, 128]],
                    base=0,
                    channel_multiplier=-1,
                    compare_op=mybir.AluOpType.is_ge,
                    fill=0.0)

        def ctx_mm(pair, r, kb, h2, ctx_ps, probs, nch):
            f0 = max(0, kb * 128 - r * RNG)
            h = 2 * pair + h2
            nc.tensor.matmul(
                ctx_ps[:, f0:],
                vh_sb[:, kb, h * (dh + 1):(h + 1) * (dh + 1)],
                probs[:, f0:],
                start=kb == 0, stop=kb == nch - 1)

        for pair in range(cfg.PAIRS):
            for r in range(cfg.NR):
                nch = min(((r + 1) * RNG) // 128, cfg.KB_MAX)
                ctx_ps = [psum.tile([dh + 1, RNG], F32, tag="ctx",
                                    bufs=2, name=f"ctx_ps{h2}")
                          for h2 in range(2)]
                prev = None
                for kb in range(nch):
                    sc = [psum.tile([128, RNG], F32, tag="sc", bufs=3,
                                    name=f"sc{h2}") for h2 in range(2)]
                    probs = [att.tile([128, RNG], BF16, tag=f"pr{h2}",
                                      bufs=3, name=f"probs{h2}")
                             for h2 in range(2)]
                    # software pipeline: ctx of kb-1 interleaves between the
                    # two score matmuls of kb so the PE never waits on exp.
                    score_mm(pair, r, kb, 0, sc[0], probs[0])
                    if prev is not None:
                        ctx_mm(pair, r, kb - 1, 1, ctx_ps[1], prev[1], nch)
                    score_mm(pair, r, kb, 1, sc[1], probs[1])
                    if prev is not None:
                        ctx_mm(pair, r, kb - 1, 0, ctx_ps[0], prev[0], nch)
                    prev = probs
                ctx_mm(pair, r, nch - 1, 0, ctx_ps[0], prev[0], nch)
                ctx_mm(pair, r, nch - 1, 1, ctx_ps[1], prev[1], nch)

                # epilogue: copy psum ctx to SBUF (frees the bank), then
                # divide rows 0..dh-1 by row dh (the prob sum) via a cheap
                # [1,R] reciprocal + partition broadcast + multiply.
                stage = att.tile([128, RNG], BF16, tag="stage")
                for h2 in range(2):
                    csb = small.tile([dh + 1, RNG], F32, tag=f"csb{h2}",
                                     name=f"csb{h2}")
                    nc.vector.tensor_copy(out=csb, in_=ctx_ps[h2])
                    rec = small.tile([1, RNG], F32, tag=f"rec{h2}",
                                     name=f"rec{h2}")
                    nc.vector.reciprocal(rec, csb[dh:dh + 1, :])
                    rbc = small.tile([64, RNG], F32, tag=f"rbc{h2}",
                                     name=f"rbc{h2}")
                    nc.gpsimd.partition_broadcast(rbc, rec)
                    nc.vector.tensor_mul(
                        stage[64 * h2:64 * h2 + 64, :],
                        csb[0:dh, :], rbc)
                # stage rows r*RNG+[0,RNG) as two A2A slots of RSL rows
                for j in range(2):
                    nc.sync.dma_start(
                        out=a2a_in[pair][2 * r + j, :, :],
                        in_=stage[:, j * RSL:(j + 1) * RSL])
            nc.gpsimd.collective_compute(
                "AllToAll", mybir.AluOpType.bypass,
                replica_groups=[list(range(cfg.NC))],
                ins=[a2a_in[pair][:]], outs=[a2a_out[pair][:]])

        # fetch gathered ctx chunks: ccb[(pair, sender)] = sender's 2 heads
        # (128 dims) of pair `pair`, for my RQ rows (RSL per batch half).
        ccb = {}
        for pair in range(cfg.PAIRS):
            for s in range(cfg.NC):
                t_ccb = ctxf.tile([128, RSL], BF16, name=f"ccb_{pair}_{s}",
                                  tag=f"ccb_{pair}_{s}")
                nc.sync.dma_start(out=t_ccb, in_=a2a_out[pair][s, :, :])
                ccb[(pair, s)] = t_ccb

        # ---- P3: Wo + residual + LayerNorm ---------------------------------
        # row-tile t covers my rows [t*128,(t+1)*128): batch b = t//2,
        # in-slot column range (t%2)*128. Sender 4b+s holds head chunk
        # (pair, s) for that batch. Pair-0 chunks are accumulated first so
        # they can proceed while the pair-1 collective is still in flight.
        for t in range(RQ // 128):
            b = t // 2
            col = slice((t % 2) * 128, (t % 2) * 128 + 128)
            pso = [psum.tile([128, cfg.WONW], F32, tag=("pj", "sc")[nsl],
                             bufs=(2, 3)[nsl], name=f"pso{nsl}")
                   for nsl in range(cfg.WON)]
            n_jc = cfg.G * cfg.D4C
            jc = 0
            for pair in range(cfg.PAIRS):
                for s in range(cfg.G):
                    cc = ccb[(pair, 4 * b + s)][:, col]
                    # global output dim chunk for (sender s, pair):
                    oc = s * cfg.D4C + pair
                    for nsl in range(cfg.WON):
                        nc.tensor.matmul(
                            pso[nsl], cc,
                            wo_sb[:, oc, nsl * cfg.WONW:(nsl + 1) * cfg.WONW],
                            start=jc == 0, stop=jc == n_jc - 1)
                    jc += 1
            x = lnp.tile([128, D], F32, tag="x")
            for nsl in range(cfg.WON):
                sl = slice(nsl * cfg.WONW, (nsl + 1) * cfg.WONW)
                nc.vector.tensor_add(x[:, sl], pso[nsl], res_sb[:, t, sl])
            fmax = math.gcd(nc.vector.BN_STATS_FMAX, D)
            nsub = D // fmax
            stats = lnp.tile([128, nsub, nc.vector.BN_STATS_DIM], F32,
                             tag="stats")
            for sg in range(nsub):
                nc.vector.bn_stats(
                    out=stats[:, sg, :],
                    in_=x.rearrange("p (a b) -> p a b", a=nsub)[:, sg, :])
            mv = lnp.tile([128, nc.vector.BN_AGGR_DIM], F32, tag="mv")
            nc.vector.bn_aggr(out=mv, in_=stats)
            sd = lnp.tile([128, 1], F32, tag="sd")
            nc.scalar.activation(out=sd, in_=mv[:, 1:2],
                                 func=mybir.ActivationFunctionType.Sqrt,
                                 bias=eps_sb, scale=1.0)
            rstd = lnp.tile([128, 1], F32, tag="rstd")
            nc.vector.reciprocal(rstd, sd)
            y = lnp.tile([128, D], F32, tag="y")
            nc.vector.tensor_scalar(
                out=y, in0=x, scalar1=mv[:, 0:1], scalar2=rstd,
                op0=mybir.AluOpType.subtract, op1=mybir.AluOpType.mult)
            yg = lnp.tile([128, D], F32, tag="yg")
            nc.vector.tensor_mul(yg, y, gamma_bc)
            out_sb = lnp.tile([128, D], F32, tag="out_sb")
            nc.vector.tensor_add(out_sb, yg, beta_bc)
            nc.sync.dma_start(out=out_shard[t * 128:(t + 1) * 128, :],
                              in_=out_sb)

    nc.compile()
    return nc


def make_in_maps(cfg: Cfg, q, k, v, Wq, Wk, Wv, Wo, gamma, beta, sen_len):
    """Host-side sharding: slice/transpose/cast per core."""
    bf = ml_dtypes.bfloat16
    in_maps = []
    woT_full = np.ascontiguousarray(Wo.T.astype(bf))
    pos = np.arange(cfg.S)
    per_batch = {}
    for b in range(cfg.B):
        per_batch[b] = (
            np.ascontiguousarray(q[b].T.astype(bf)),
            np.ascontiguousarray(k[b].T.astype(bf)),
            np.ascontiguousarray(v[b].T.astype(bf)),
            np.where(pos < int(sen_len[b]), 0.0, NEG_INF).astype(np.float32),
        )
    for c in range(cfg.NC):
        g = c // cfg.G
        l = c % cfg.G
        hs = slice(l * cfg.D4, (l + 1) * cfg.D4)
        rows = slice(c * cfg.RSL, (c + 1) * cfg.RSL)
        qTb, kTb, vTb, pb = per_batch[g]
        res = np.concatenate([q[b, rows, :] for b in range(cfg.B)], axis=0)
        in_maps.append({
            "qT": qTb, "kT": kTb, "vT": vTb,
            "wqT": np.ascontiguousarray(Wq[hs, :].T.astype(bf)),
            "wkT": np.ascontiguousarray(Wk[hs, :].T.astype(bf)),
            "wvT": np.ascontiguousarray(Wv[hs, :].T.astype(bf)),
            "woT": woT_full,
            "resid": np.ascontiguousarray(res).astype(np.float32),
            "pad_bias": pb.reshape(cfg.KCH, 128),
            "gamma": gamma.reshape(1, cfg.D).astype(np.float32),
            "beta": beta.reshape(1, cfg.D).astype(np.float32),
        })
    return in_maps


def assemble_output(cfg: Cfg, results):
    out = np.empty((cfg.B, cfg.S, cfg.D), np.float32)
    for c in range(cfg.NC):
        rows = slice(c * cfg.RSL, (c + 1) * cfg.RSL)
        for b in range(cfg.B):
            out[b, rows, :] = results[c]["out_shard"][
                b * cfg.RSL:(b + 1) * cfg.RSL]
    return out


_PROGRAM_CACHE = {}


def _get_program(cfg: Cfg):
    key = (cfg.B, cfg.S, cfg.D, cfg.H, cfg.dh, cfg.KB_MAX)
    if key not in _PROGRAM_CACHE:
        _PROGRAM_CACHE[key] = build_program(cfg)
    return _PROGRAM_CACHE[key]


def run(cfg: Cfg, inputs: dict, trace: bool = False):
    nc = _get_program(cfg)
    in_maps = make_in_maps(cfg, **inputs)
    res = run_bass_kernel_spmd(nc, in_maps, core_ids=list(range(cfg.NC)),
                               trace=trace)
    return assemble_output(cfg, res.results), res


def kernel(**inputs) -> np.ndarray:
    kmax = int(np.max(inputs["sen_len"]))
    cfg = Cfg(B=2, S=2048, D=1024, H=16, dh=64, kmax=kmax)
    out, _ = run(cfg, inputs)
    return out


# revision 6
# speedup vs baseline: 1.4276x; 1.0742x over previous
"""Multi-head attention (projections + causal/padded softmax attention + output
projection + residual + LayerNorm) as a Bass/Tile kernel on 8 Trainium2 cores.

Sharding: tensor-parallel over heads within each batch. Core c handles batch
g = c // 4 and heads [4*(c%4), 4*(c%4)+4). Each core projects Q/K/V for its
4 heads over the full sequence, runs causal attention in a transposed layout
(scoresT[key, row]), and produces ctxT[dh, row]. One 8-way AllToAll per
head-pair redistributes ctxT with a fully STATIC slot map: slot j carries rows
[j*256, (j+1)*256) of the sender's batch, so core j ends up owning that row
range of BOTH batches (cores 0-3 receive batch-0 contributions from cores 0-3
and batch-1 contributions from cores 4-7 in distinct sender slots). No runtime
core-id addressing, no barriers: Tile orders staging DMAs before each
collective and the pair-0 collective overlaps pair-1's attention.

Layout trick: all matmul operands are pre-transposed/pre-cast on the host
(numpy) so every DMA is contiguous: qT/kT/vT = x^T as bf16, WqT/WkT/WvT/WoT =
W^T as bf16. The PE contracts over partitions, so the contraction dim (d_model
or d_head) always sits on the partition axis.

Softmax: scores are bounded (|s| ~ 5) so exp is computed without max
subtraction; exp(scale*s + pad_bias) runs on the scalar engine with the
padding mask folded into the per-key bias. The causal boundary is enforced by
zeroing probs with gpsimd.affine_select. The denominator is obtained by
augmenting V with a ones column (row 64 of ctxT psum = sum of probs).

Attention is software-pipelined for the PE p-state ramp: the ctx matmul of
chunk kb is emitted between the score matmuls of chunk kb+1 so the tensor
engine never waits on the scalar-engine exp. The softmax epilogue first copies
the ctx psum to SBUF (freeing the bank), then does a cheap [1,R] reciprocal,
partition-broadcast and multiply off the critical path.

PSUM budget (8 banks): pj=2 + sc=3 + ctx=2 = 7 (pj/sc shapes are reused for
the Wo accumulators in P3).
"""

import math
from contextlib import ExitStack

import numpy as np
import ml_dtypes

import concourse.bass as bass
import concourse.mybir as mybir
import concourse.tile as tile
from concourse import bacc
from concourse.bass_utils import run_bass_kernel_spmd

BF16 = mybir.dt.bfloat16
F32 = mybir.dt.float32

NEG_INF = -1e9
LN_EPS = 1e-6


class Cfg:
    def __init__(self, B=2, S=2048, D=1024, H=16, dh=64, kmax=None):
        self.B, self.S, self.D, self.H, self.dh = B, S, D, H, dh
        # kmax: max(sen_len) — keys beyond are fully masked, so K/V
        # projection and the attention key loop stop at this bound.
        self.kmax = S if kmax is None else min(int(kmax), S)
        self.NC = 8                      # cores
        self.G = 4                       # cores per batch group
        self.HPC = H // self.G           # heads per core
        self.PAIRS = self.HPC // 2       # head pairs per core
        self.D4 = self.HPC * dh          # per-core projection width
        self.RQ = S // self.G            # rows per core in Wo/LN phase
        self.NR = 4                      # attention row ranges
        self.RNG = S // self.NR          # rows per range (== RQ)
        self.RSL = S // self.NC          # rows per A2A slot (256)
        self.DC = D // 128               # contraction chunks
        self.KCH = S // 128              # key chunks
        self.NS = max(1, S // 512)       # projection n-slices
        self.NSW = S // self.NS          # cols per n-slice
        self.WON = max(1, D // 512)      # Wo n-slices
        self.WONW = D // self.WON
        self.D4C = self.D4 // 128        # 128-chunks in per-core ctx width
        self.KB_MAX = -(-self.kmax // 128)          # key chunks actually used
        self.NS_K = -(-(self.KB_MAX * 128) // self.NSW)  # K-proj n-slices
        assert self.RQ == self.RNG
        assert self.PAIRS >= 1 and self.HPC % 2 == 0


def build_program(cfg: Cfg):
    """Build the (SPMD-identical) Bass program."""
    nc = bacc.Bacc("TRN2", target_bir_lowering=False, debug=False,
                   num_devices=cfg.NC)

    S, D, dh = cfg.S, cfg.D, cfg.dh
    D4, RQ, RNG, RSL = cfg.D4, cfg.RQ, cfg.RNG, cfg.RSL

    qT = nc.dram_tensor("qT", [D, S], BF16, kind="ExternalInput").ap()
    kT = nc.dram_tensor("kT", [D, S], BF16, kind="ExternalInput").ap()
    vT = nc.dram_tensor("vT", [D, S], BF16, kind="ExternalInput").ap()
    wqT = nc.dram_tensor("wqT", [D, D4], BF16, kind="ExternalInput").ap()
    wkT = nc.dram_tensor("wkT", [D, D4], BF16, kind="ExternalInput").ap()
    wvT = nc.dram_tensor("wvT", [D, D4], BF16, kind="ExternalInput").ap()
    woT = nc.dram_tensor("woT", [D, D], BF16, kind="ExternalInput").ap()
    resid = nc.dram_tensor("resid", [RQ, D], F32, kind="ExternalInput").ap()
    pad_bias = nc.dram_tensor("pad_bias", [cfg.KCH, 128], F32,
                              kind="ExternalInput").ap()
    gamma = nc.dram_tensor("gamma", [1, D], F32, kind="ExternalInput").ap()
    beta = nc.dram_tensor("beta", [1, D], F32, kind="ExternalInput").ap()
    out_shard = nc.dram_tensor("out_shard", [RQ, D], F32,
                               kind="ExternalOutput").ap()

    with tile.TileContext(nc) as tc, ExitStack() as ctx:
        consts = ctx.enter_context(tc.tile_pool(name="consts", bufs=1))
        xin = ctx.enter_context(tc.tile_pool(name="xin", bufs=2))
        proj = ctx.enter_context(tc.tile_pool(name="proj", bufs=1))
        att = ctx.enter_context(tc.tile_pool(name="att", bufs=2))
        small = ctx.enter_context(tc.tile_pool(name="small", bufs=2))
        lnp = ctx.enter_context(tc.tile_pool(name="lnp", bufs=2))
        ctxf = ctx.enter_context(tc.tile_pool(name="ctxf", bufs=1))
        dram = ctx.enter_context(
            tc.tile_pool(name="dram", bufs=1, space="DRAM"))
        psum = ctx.enter_context(
            tc.tile_pool(name="psum", bufs=1, space="PSUM"))

        # ---- prologue: all constants (incl. P3's, so P3 never waits) -------
        wq_sb = consts.tile([128, cfg.DC, D4], BF16)
        wk_sb = consts.tile([128, cfg.DC, D4], BF16)
        wv_sb = consts.tile([128, cfg.DC, D4], BF16)
        for w_sb, w_dram in ((wk_sb, wkT), (wv_sb, wvT), (wq_sb, wqT)):
            nc.sync.dma_start(
                out=w_sb, in_=w_dram.rearrange("(c p) o -> p c o", p=128))

        pb_sb = consts.tile([128, cfg.KCH], F32)
        nc.sync.dma_start(out=pb_sb, in_=pad_bias.rearrange("c p -> p c"))

        # P3 constants stream on the scalar engine's DMA queue so they don't
        # delay P1's input stream on the sync queue (scalar is idle in P1).
        wo_sb = consts.tile([128, cfg.DC, D], BF16)
        nc.scalar.dma_start(out=wo_sb,
                            in_=woT.rearrange("(c p) o -> p c o", p=128))
        g_row = consts.tile([1, D], F32)
        b_row = consts.tile([1, D], F32)
        nc.scalar.dma_start(out=g_row, in_=gamma)
        nc.scalar.dma_start(out=b_row, in_=beta)
        gamma_bc = consts.tile([128, D], F32)
        beta_bc = consts.tile([128, D], F32)
        nc.gpsimd.partition_broadcast(gamma_bc, g_row)
        nc.gpsimd.partition_broadcast(beta_bc, b_row)
        eps_sb = consts.tile([128, 1], F32)
        nc.vector.memset(eps_sb, LN_EPS)
        res_sb = consts.tile([128, cfg.G, D], F32)
        nc.scalar.dma_start(
            out=res_sb, in_=resid.rearrange("(t p) d -> p t d", p=128))

        # A2A buffers: one per head-pair; slot j = rows [j*RSL,(j+1)*RSL).
        a2a_in = [dram.tile([cfg.NC, 128, RSL], BF16, name=f"a2a_in{p}",
                            tag=f"a2a_in{p}") for p in range(cfg.PAIRS)]
        a2a_out = [dram.tile([cfg.NC, 128, RSL], BF16, name=f"a2a_out{p}",
                             tag=f"a2a_out{p}") for p in range(cfg.PAIRS)]

        # warm up the NRT collective stream during P1: the first collective
        # after the prelude barrier pays ~11us of trigger latency; a dummy
        # 4KB AllToAll absorbs it so cc0/cc1 start promptly.
        warm_in = dram.tile([cfg.NC, 128, 2], BF16, name="warm_in",
                            tag="warm_in")
        warm_out = dram.tile([cfg.NC, 128, 2], BF16, name="warm_out",
                             tag="warm_out")
        nc.gpsimd.collective_compute(
            "AllToAll", mybir.AluOpType.bypass,
            replica_groups=[list(range(cfg.NC))],
            ins=[warm_in[:]], outs=[warm_out[:]])

        # ---- P1: projections (K, V first so attention can start early) ----
        qhT_sb = proj.tile([128, cfg.PAIRS, S], BF16)
        khT_sb = proj.tile([128, cfg.PAIRS, S], BF16)
        vh_sb = proj.tile([128, cfg.KCH, cfg.HPC * (dh + 1)], BF16)

        def qk_proj(x_dram, w_sb, out_sb, ns_count=None):
            for ns in range(ns_count if ns_count is not None else cfg.NS):
                x_ns = xin.tile([128, cfg.DC, cfg.NSW], BF16, tag="x_ns",
                                name="x_ns")
                nc.sync.dma_start(
                    out=x_ns, in_=x_dram.rearrange("(c p) s -> p c s", p=128)
                    [:, :, ns * cfg.NSW:(ns + 1) * cfg.NSW])
                for pair in range(cfg.PAIRS):
                    ps = psum.tile([128, cfg.NSW], F32, tag="pj", bufs=2,
                                   name="ps_pj")
                    for dc in range(cfg.DC):
                        nc.tensor.matmul(
                            ps, w_sb[:, dc, pair * 128:(pair + 1) * 128],
                            x_ns[:, dc, :],
                            start=dc == 0, stop=dc == cfg.DC - 1)
                    nc.vector.tensor_copy(
                        out=out_sb[:, pair, ns * cfg.NSW:(ns + 1) * cfg.NSW],
                        in_=ps)

        qk_proj(kT, wk_sb, khT_sb, ns_count=cfg.NS_K)

        for kb in range(cfg.KB_MAX):
            v_kb = xin.tile([128, cfg.DC, 128], BF16, tag="v_kb")
            nc.sync.dma_start(
                out=v_kb, in_=vT.rearrange("(c p) s -> p c s", p=128)
                [:, :, kb * 128:(kb + 1) * 128])
            psv = psum.tile([128, D4], F32, tag="pj", bufs=2, name="ps_v")
            for dc in range(cfg.DC):
                nc.tensor.matmul(psv, v_kb[:, dc, :], wv_sb[:, dc, :],
                                 start=dc == 0, stop=dc == cfg.DC - 1)
            nc.vector.tensor_copy(
                out=vh_sb[:, kb, :].rearrange("p (h e) -> p h e", e=dh + 1)
                [:, :, 0:dh],
                in_=psv.rearrange("p (h e) -> p h e", e=dh))
            nc.vector.memset(
                vh_sb[:, kb, :].rearrange("p (h e) -> p h e", e=dh + 1)
                [:, :, dh:dh + 1], 1.0)

        qk_proj(qT, wq_sb, qhT_sb)

        # ---- P2: attention; per-pair A2A overlaps the next pair -----------
        def score_mm(pair, r, kb, h2, sc, probs):
            """Score matmul + exp (+ causal select) for one chunk/head."""
            f0 = max(0, kb * 128 - r * RNG)
            w = RNG - f0
            lo, hi = 64 * h2, 64 * h2 + 64
            nc.tensor.matmul(
                sc[:, 0:w],
                khT_sb[lo:hi, pair, kb * 128:(kb + 1) * 128],
                qhT_sb[lo:hi, pair, r * RNG + f0:(r + 1) * RNG],
                start=True, stop=True)
            nc.scalar.activation(
                out=probs[:, f0:], in_=sc[:, 0:w],
                func=mybir.ActivationFunctionType.Exp,
                bias=pb_sb[:, kb:kb + 1],
                scale=1.0 / math.sqrt(dh))
            if f0 > 0 or kb * 128 == r * RNG:
                # partial band: keep f - f0 >= p
                nc.gpsimd.affine_select(
                    out=probs[:, f0:f0 + 128],
                    in_=probs[:, f0:f0 + 128],
                    pattern=[[1, 128]],
                    base=0,
                    channel_multiplier=-1,
                    compare_op=mybir.AluOpType.is_ge,
                    fill=0.0)

        def ctx_mm(pair, r, kb, h2, ctx_ps, probs, nch):
            f0 = max(0, kb * 128 - r * RNG)
            h = 2 * pair + h2
            nc.tensor.matmul(
                ctx_ps[:, f0:],
                vh_sb[:, kb, h * (dh + 1):(h + 1) * (dh + 1)],
                probs[:, f0:],
                start=kb == 0, stop=kb == nch - 1)

        for pair in range(cfg.PAIRS):
            for r in range(cfg.NR):
                nch = min(((r + 1) * RNG) // 128, cfg.KB_MAX)
                ctx_ps = [psum.tile([dh + 1, RNG], F32, tag="ctx",
                                    bufs=2, name=f"ctx_ps{h2}")
                          for h2 in range(2)]
                prev = None
                for kb in range(nch):
                    sc = [psum.tile([128, RNG], F32, tag="sc", bufs=3,
                                    name=f"sc{h2}") for h2 in range(2)]
                    probs = [att.tile([128, RNG], BF16, tag=f"pr{h2}",
                                      bufs=3, name=f"probs{h2}")
                             for h2 in range(2)]
                    # software pipeline: ctx of kb-1 interleaves between the
                    # two score matmuls of kb so the PE never waits on exp.
                    score_mm(pair, r, kb, 0, sc[0], probs[0])
                    if prev is not None:
                        ctx_mm(pair, r, kb - 1, 1, ctx_ps[1], prev[1], nch)
                    score_mm(pair, r, kb, 1, sc[1], probs[1])
                    if prev is not None:
                        ctx_mm(pair, r, kb - 1, 0, ctx_ps[0], prev[0], nch)
                    prev = probs
                ctx_mm(pair, r, nch - 1, 0, ctx_ps[0], prev[0], nch)
                ctx_mm(pair, r, nch - 1, 1, ctx_ps[1], prev[1], nch)

                # epilogue: copy psum ctx to SBUF (frees the bank), then
                # divide rows 0..dh-1 by row dh (the prob sum) via a cheap
                # [1,R] reciprocal + partition broadcast + multiply.
                stage = att.tile([128, RNG], BF16, tag="stage", bufs=4)
                for h2 in range(2):
                    csb = small.tile([dh + 1, RNG], F32, tag=f"csb{h2}",
                                     name=f"csb{h2}")
                    nc.vector.tensor_copy(out=csb, in_=ctx_ps[h2])
                    rec = small.tile([1, RNG], F32, tag=f"rec{h2}",
                                     name=f"rec{h2}")
                    nc.vector.reciprocal(rec, csb[dh:dh + 1, :])
                    rbc = small.tile([64, RNG], F32, tag=f"rbc{h2}",
                                     name=f"rbc{h2}")
                    nc.gpsimd.partition_broadcast(rbc, rec)
                    nc.vector.tensor_mul(
                        stage[64 * h2:64 * h2 + 64, :],
                        csb[0:dh, :], rbc)
                # stage rows r*RNG+[0,RNG) as two A2A slots of RSL rows
                for j in range(2):
                    nc.sync.dma_start(
                        out=a2a_in[pair][2 * r + j, :, :],
                        in_=stage[:, j * RSL:(j + 1) * RSL])
            nc.gpsimd.collective_compute(
                "AllToAll", mybir.AluOpType.bypass,
                replica_groups=[list(range(cfg.NC))],
                ins=[a2a_in[pair][:]], outs=[a2a_out[pair][:]])

        # Scheduler fence (no runtime syncs): without it the scheduler hoists
        # the cc0-gated fetch DMAs ahead of pair-1's staging DMAs on the sync
        # queue, stalling the whole queue until cc0 completes.
        tc.no_sync_barrier()

        # fetch gathered ctx chunks: ccb[(pair, sender)] = sender's 2 heads
        # (128 dims) of pair `pair`, for my RQ rows (RSL per batch half).
        ccb = {}
        for pair in range(cfg.PAIRS):
            for s in range(cfg.NC):
                t_ccb = ctxf.tile([128, RSL], BF16, name=f"ccb_{pair}_{s}",
                                  tag=f"ccb_{pair}_{s}")
                nc.sync.dma_start(out=t_ccb, in_=a2a_out[pair][s, :, :])
                ccb[(pair, s)] = t_ccb

        # ---- P3: Wo + residual + LayerNorm ---------------------------------
        # row-tile t covers my rows [t*128,(t+1)*128): batch b = t//2,
        # in-slot column range (t%2)*128. Sender 4b+s holds head chunk
        # (pair, s) for that batch. Pair-0 chunks are accumulated first so
        # they can proceed while the pair-1 collective is still in flight.
        for t in range(RQ // 128):
            b = t // 2
            col = slice((t % 2) * 128, (t % 2) * 128 + 128)
            pso = [psum.tile([128, cfg.WONW], F32, tag=("pj", "sc")[nsl],
                             bufs=(2, 3)[nsl], name=f"pso{nsl}")
                   for nsl in range(cfg.WON)]
            n_jc = cfg.G * cfg.D4C
            jc = 0
            for pair in range(cfg.PAIRS):
                for s in range(cfg.G):
                    cc = ccb[(pair, 4 * b + s)][:, col]
                    # global output dim chunk for (sender s, pair):
                    oc = s * cfg.D4C + pair
                    for nsl in range(cfg.WON):
                        nc.tensor.matmul(
                            pso[nsl], cc,
                            wo_sb[:, oc, nsl * cfg.WONW:(nsl + 1) * cfg.WONW],
                            start=jc == 0, stop=jc == n_jc - 1)
                    jc += 1
            x = lnp.tile([128, D], F32, tag="x")
            for nsl in range(cfg.WON):
                sl = slice(nsl * cfg.WONW, (nsl + 1) * cfg.WONW)
                nc.vector.tensor_add(x[:, sl], pso[nsl], res_sb[:, t, sl])
            fmax = math.gcd(nc.vector.BN_STATS_FMAX, D)
            nsub = D // fmax
            stats = lnp.tile([128, nsub, nc.vector.BN_STATS_DIM], F32,
                             tag="stats")
            for sg in range(nsub):
                nc.vector.bn_stats(
                    out=stats[:, sg, :],
                    in_=x.rearrange("p (a b) -> p a b", a=nsub)[:, sg, :])
            mv = lnp.tile([128, nc.vector.BN_AGGR_DIM], F32, tag="mv")
            nc.vector.bn_aggr(out=mv, in_=stats)
            sd = lnp.tile([128, 1], F32, tag="sd")
            nc.scalar.activation(out=sd, in_=mv[:, 1:2],
                                 func=mybir.ActivationFunctionType.Sqrt,
                                 bias=eps_sb, scale=1.0)
            rstd = lnp.tile([128, 1], F32, tag="rstd")
            nc.vector.reciprocal(rstd, sd)
            y = lnp.tile([128, D], F32, tag="y")
            nc.vector.tensor_scalar(
                out=y, in0=x, scalar1=mv[:, 0:1], scalar2=rstd,
                op0=mybir.AluOpType.subtract, op1=mybir.AluOpType.mult)
            yg = lnp.tile([128, D], F32, tag="yg")
            nc.vector.tensor_mul(yg, y, gamma_bc)
            out_sb = lnp.tile([128, D], F32, tag="out_sb")
            nc.vector.tensor_add(out_sb, yg, beta_bc)
            nc.sync.dma_start(out=out_shard[t * 128:(t + 1) * 128, :],
                              in_=out_sb)

    nc.compile()
    return nc


def make_in_maps(cfg: Cfg, q, k, v, Wq, Wk, Wv, Wo, gamma, beta, sen_len):
    """Host-side sharding: slice/transpose/cast per core."""
    bf = ml_dtypes.bfloat16
    in_maps = []
    woT_full = np.ascontiguousarray(Wo.T.astype(bf))
    pos = np.arange(cfg.S)
    per_batch = {}
    for b in range(cfg.B):
        per_batch[b] = (
            np.ascontiguousarray(q[b].T.astype(bf)),
            np.ascontiguousarray(k[b].T.astype(bf)),
            np.ascontiguousarray(v[b].T.astype(bf)),
            np.where(pos < int(sen_len[b]), 0.0, NEG_INF).astype(np.float32),
        )
    for c in range(cfg.NC):
        g = c // cfg.G
        l = c % cfg.G
        hs = slice(l * cfg.D4, (l + 1) * cfg.D4)
        rows = slice(c * cfg.RSL, (c + 1) * cfg.RSL)
        qTb, kTb, vTb, pb = per_batch[g]
        res = np.concatenate([q[b, rows, :] for b in range(cfg.B)], axis=0)
        in_maps.append({
            "qT": qTb, "kT": kTb, "vT": vTb,
            "wqT": np.ascontiguousarray(Wq[hs, :].T.astype(bf)),
            "wkT": np.ascontiguousarray(Wk[hs, :].T.astype(bf)),
            "wvT": np.ascontiguousarray(Wv[hs, :].T.astype(bf)),
            "woT": woT_full,
            "resid": np.ascontiguousarray(res).astype(np.float32),
            "pad_bias": pb.reshape(cfg.KCH, 128),
            "gamma": gamma.reshape(1, cfg.D).astype(np.float32),
            "beta": beta.reshape(1, cfg.D).astype(np.float32),
        })
    return in_maps


def assemble_output(cfg: Cfg, results):
    out = np.empty((cfg.B, cfg.S, cfg.D), np.float32)
    for c in range(cfg.NC):
        rows = slice(c * cfg.RSL, (c + 1) * cfg.RSL)
        for b in range(cfg.B):
            out[b, rows, :] = results[c]["out_shard"][
                b * cfg.RSL:(b + 1) * cfg.RSL]
    return out


_PROGRAM_CACHE = {}


def _get_program(cfg: Cfg):
    key = (cfg.B, cfg.S, cfg.D, cfg.H, cfg.dh, cfg.KB_MAX)
    if key not in _PROGRAM_CACHE:
        _PROGRAM_CACHE[key] = build_program(cfg)
    return _PROGRAM_CACHE[key]


def run(cfg: Cfg, inputs: dict, trace: bool = False):
    nc = _get_program(cfg)
    in_maps = make_in_maps(cfg, **inputs)
    res = run_bass_kernel_spmd(nc, in_maps, core_ids=list(range(cfg.NC)),
                               trace=trace)
    return assemble_output(cfg, res.results), res


def kernel(**inputs) -> np.ndarray:
    kmax = int(np.max(inputs["sen_len"]))
    cfg = Cfg(B=2, S=2048, D=1024, H=16, dh=64, kmax=kmax)
    out, _ = run(cfg, inputs)
    return out


# revision 13
# speedup vs baseline: 1.4849x; 1.0402x over previous
"""Multi-head attention (projections + causal/padded softmax attention + output
projection + residual + LayerNorm) as a Bass/Tile kernel on 8 Trainium2 cores.

Sharding: tensor-parallel over heads within each batch. Core c handles batch
g = c // 4 and heads [4*(c%4), 4*(c%4)+4). Each core projects Q/K/V for its
4 heads over the full sequence, runs causal attention in a transposed layout
(scoresT[key, row]), and produces ctxT[dh, row]. One 8-way AllToAll per
head-pair redistributes ctxT with a fully STATIC slot map: slot j carries rows
[j*256, (j+1)*256) of the sender's batch, so core j ends up owning that row
range of BOTH batches (cores 0-3 receive batch-0 contributions from cores 0-3
and batch-1 contributions from cores 4-7 in distinct sender slots). No runtime
core-id addressing, no barriers: Tile orders staging DMAs before each
collective and the pair-0 collective overlaps pair-1's attention.

Layout trick: all matmul operands are pre-transposed/pre-cast on the host
(numpy) so every DMA is contiguous: qT/kT/vT = x^T as bf16, WqT/WkT/WvT/WoT =
W^T as bf16. The PE contracts over partitions, so the contraction dim (d_model
or d_head) always sits on the partition axis.

Softmax: scores are bounded (|s| ~ 5) so exp is computed without max
subtraction; exp(scale*s + pad_bias) runs on the scalar engine with the
padding mask folded into the per-key bias. The causal boundary is enforced by
zeroing probs with gpsimd.affine_select. The denominator is obtained by
augmenting V with a ones column (row 64 of ctxT psum = sum of probs).

Attention is software-pipelined for the PE p-state ramp: the ctx matmul of
chunk kb is emitted between the score matmuls of chunk kb+1 so the tensor
engine never waits on the scalar-engine exp. The softmax epilogue first copies
the ctx psum to SBUF (freeing the bank), then does a cheap [1,R] reciprocal,
partition-broadcast and multiply off the critical path.

PSUM budget (8 banks): pj=2 + sc=3 + ctx=2 = 7 (pj/sc shapes are reused for
the Wo accumulators in P3).
"""

import math
from contextlib import ExitStack

import numpy as np
import ml_dtypes

import concourse.bass as bass
import concourse.mybir as mybir
import concourse.tile as tile
from concourse import bacc
from concourse.bass_utils import run_bass_kernel_spmd

BF16 = mybir.dt.bfloat16
F32 = mybir.dt.float32

NEG_INF = -1e9
LN_EPS = 1e-6


class Cfg:
    def __init__(self, B=2, S=2048, D=1024, H=16, dh=64, kmax=None):
        self.B, self.S, self.D, self.H, self.dh = B, S, D, H, dh
        # kmax: max(sen_len) — keys beyond are fully masked, so K/V
        # projection and the attention key loop stop at this bound.
        self.kmax = S if kmax is None else min(int(kmax), S)
        self.NC = 8                      # cores
        self.G = 4                       # cores per batch group
        self.HPC = H // self.G           # heads per core
        self.PAIRS = self.HPC // 2       # head pairs per core
        self.D4 = self.HPC * dh          # per-core projection width
        self.RQ = S // self.G            # rows per core in Wo/LN phase
        self.NR = 4                      # attention row ranges
        self.RNG = S // self.NR          # rows per range (== RQ)
        self.RSL = S // self.NC          # rows per A2A slot (256)
        self.DC = D // 128               # contraction chunks
        self.KCH = S // 128              # key chunks
        self.NS = max(1, S // 512)       # projection n-slices
        self.NSW = S // self.NS          # cols per n-slice
        self.WON = max(1, D // 512)      # Wo n-slices
        self.WONW = D // self.WON
        self.D4C = self.D4 // 128        # 128-chunks in per-core ctx width
        self.KB_MAX = -(-self.kmax // 128)          # key chunks actually used
        self.NS_K = -(-(self.KB_MAX * 128) // self.NSW)  # K-proj n-slices
        assert self.RQ == self.RNG
        assert self.PAIRS >= 1 and self.HPC % 2 == 0


def build_program(cfg: Cfg):
    """Build the (SPMD-identical) Bass program."""
    nc = bacc.Bacc("TRN2", target_bir_lowering=False, debug=False,
                   num_devices=cfg.NC)

    S, D, dh = cfg.S, cfg.D, cfg.dh
    D4, RQ, RNG, RSL = cfg.D4, cfg.RQ, cfg.RNG, cfg.RSL

    # All inputs are pre-tiled on the host so every DMA is contiguous per
    # partition: x inputs as [ns, p, dc, cols], weights as [p, dc, outs].
    qT = nc.dram_tensor("qT", [cfg.NS, 128, cfg.DC, cfg.NSW], BF16,
                        kind="ExternalInput").ap()
    kT = nc.dram_tensor("kT", [cfg.NS_K, 128, cfg.DC, cfg.NSW], BF16,
                        kind="ExternalInput").ap()
    vT = nc.dram_tensor("vT", [cfg.KB_MAX, 128, cfg.DC, 128], BF16,
                        kind="ExternalInput").ap()
    wqT = nc.dram_tensor("wqT", [128, cfg.DC, D4], BF16,
                         kind="ExternalInput").ap()
    wkT = nc.dram_tensor("wkT", [128, cfg.DC, D4], BF16,
                         kind="ExternalInput").ap()
    wvT = nc.dram_tensor("wvT", [128, cfg.DC, D4], BF16,
                         kind="ExternalInput").ap()
    woT = nc.dram_tensor("woT", [128, cfg.DC, D], BF16,
                         kind="ExternalInput").ap()
    resid = nc.dram_tensor("resid", [128, cfg.G, D], F32,
                           kind="ExternalInput").ap()
    pad_bias = nc.dram_tensor("pad_bias", [128, cfg.KCH], F32,
                              kind="ExternalInput").ap()
    gamma = nc.dram_tensor("gamma", [1, D], F32, kind="ExternalInput").ap()
    beta = nc.dram_tensor("beta", [1, D], F32, kind="ExternalInput").ap()
    out_shard = nc.dram_tensor("out_shard", [RQ, D], F32,
                               kind="ExternalOutput").ap()

    with tile.TileContext(nc) as tc, ExitStack() as ctx:
        consts = ctx.enter_context(tc.tile_pool(name="consts", bufs=1))
        xin = ctx.enter_context(tc.tile_pool(name="xin", bufs=2))
        proj = ctx.enter_context(tc.tile_pool(name="proj", bufs=1))
        att = ctx.enter_context(tc.tile_pool(name="att", bufs=2))
        small = ctx.enter_context(tc.tile_pool(name="small", bufs=2))
        lnp = ctx.enter_context(tc.tile_pool(name="lnp", bufs=2))
        ctxf = ctx.enter_context(tc.tile_pool(name="ctxf", bufs=1))
        dram = ctx.enter_context(
            tc.tile_pool(name="dram", bufs=1, space="DRAM"))
        psum = ctx.enter_context(
            tc.tile_pool(name="psum", bufs=1, space="PSUM"))

        # ---- prologue: all constants (incl. P3's, so P3 never waits) -------
        wq_sb = consts.tile([128, cfg.DC, D4], BF16)
        wk_sb = consts.tile([128, cfg.DC, D4], BF16)
        wv_sb = consts.tile([128, cfg.DC, D4], BF16)
        for w_sb, w_dram in ((wk_sb, wkT), (wv_sb, wvT), (wq_sb, wqT)):
            nc.sync.dma_start(out=w_sb, in_=w_dram)

        pb_sb = consts.tile([128, cfg.KCH], F32)
        nc.sync.dma_start(out=pb_sb, in_=pad_bias)

        # P3 constants stream on the scalar engine's DMA queue so they don't
        # delay P1's input stream on the sync queue (scalar is idle in P1).
        wo_sb = consts.tile([128, cfg.DC, D], BF16)
        nc.scalar.dma_start(out=wo_sb, in_=woT)
        g_row = consts.tile([1, D], F32)
        b_row = consts.tile([1, D], F32)
        nc.scalar.dma_start(out=g_row, in_=gamma)
        nc.scalar.dma_start(out=b_row, in_=beta)
        gamma_bc = consts.tile([128, D], F32)
        beta_bc = consts.tile([128, D], F32)
        nc.gpsimd.partition_broadcast(gamma_bc, g_row)
        nc.gpsimd.partition_broadcast(beta_bc, b_row)
        eps_sb = consts.tile([128, 1], F32)
        nc.vector.memset(eps_sb, LN_EPS)
        res_sb = consts.tile([128, cfg.G, D], F32)
        nc.scalar.dma_start(out=res_sb, in_=resid)

        # A2A buffers: one per head-pair; slot j = rows [j*RSL,(j+1)*RSL).
        a2a_in = [dram.tile([cfg.NC, 128, RSL], BF16, name=f"a2a_in{p}",
                            tag=f"a2a_in{p}") for p in range(cfg.PAIRS)]
        a2a_out = [dram.tile([cfg.NC, 128, RSL], BF16, name=f"a2a_out{p}",
                             tag=f"a2a_out{p}") for p in range(cfg.PAIRS)]

        # warm up the NRT collective stream during P1: the first collective
        # after the prelude barrier pays ~11us of trigger latency; a dummy
        # 4KB AllToAll absorbs it so cc0/cc1 start promptly.
        warm_in = dram.tile([cfg.NC, 128, 2], BF16, name="warm_in",
                            tag="warm_in")
        warm_out = dram.tile([cfg.NC, 128, 2], BF16, name="warm_out",
                             tag="warm_out")
        nc.gpsimd.collective_compute(
            "AllToAll", mybir.AluOpType.bypass,
            replica_groups=[list(range(cfg.NC))],
            ins=[warm_in[:]], outs=[warm_out[:]])

        # ---- P1: projections (K, V first so attention can start early) ----
        qhT_sb = proj.tile([128, cfg.PAIRS, S], BF16)
        khT_sb = proj.tile([128, cfg.PAIRS, S], BF16)
        vh_sb = proj.tile([128, cfg.KCH, cfg.HPC * (dh + 1)], BF16)

        def qk_proj(x_dram, w_sb, out_sb, ns_count=None):
            for ns in range(ns_count if ns_count is not None else cfg.NS):
                x_ns = xin.tile([128, cfg.DC, cfg.NSW], BF16, tag="x_ns",
                                name="x_ns")
                nc.sync.dma_start(out=x_ns, in_=x_dram[ns])
                for pair in range(cfg.PAIRS):
                    ps = psum.tile([128, cfg.NSW], F32, tag="pj", bufs=2,
                                   name="ps_pj")
                    for dc in range(cfg.DC):
                        nc.tensor.matmul(
                            ps, w_sb[:, dc, pair * 128:(pair + 1) * 128],
                            x_ns[:, dc, :],
                            start=dc == 0, stop=dc == cfg.DC - 1)
                    nc.vector.tensor_copy(
                        out=out_sb[:, pair, ns * cfg.NSW:(ns + 1) * cfg.NSW],
                        in_=ps)

        qk_proj(kT, wk_sb, khT_sb, ns_count=cfg.NS_K)

        for kb in range(cfg.KB_MAX):
            v_kb = xin.tile([128, cfg.DC, 128], BF16, tag="v_kb")
            nc.sync.dma_start(out=v_kb, in_=vT[kb])
            psv = psum.tile([128, D4], F32, tag="pj", bufs=2, name="ps_v")
            for dc in range(cfg.DC):
                nc.tensor.matmul(psv, v_kb[:, dc, :], wv_sb[:, dc, :],
                                 start=dc == 0, stop=dc == cfg.DC - 1)
            nc.vector.tensor_copy(
                out=vh_sb[:, kb, :].rearrange("p (h e) -> p h e", e=dh + 1)
                [:, :, 0:dh],
                in_=psv.rearrange("p (h e) -> p h e", e=dh))
            nc.vector.memset(
                vh_sb[:, kb, :].rearrange("p (h e) -> p h e", e=dh + 1)
                [:, :, dh:dh + 1], 1.0)

        qk_proj(qT, wq_sb, qhT_sb)

        # ---- P2: attention; per-pair A2A overlaps the next pair -----------
        def score_mm(pair, r, kb, h2, sc, probs):
            """Score matmul + exp (+ causal select) for one chunk/head."""
            f0 = max(0, kb * 128 - r * RNG)
            w = RNG - f0
            lo, hi = 64 * h2, 64 * h2 + 64
            nc.tensor.matmul(
                sc[:, 0:w],
                khT_sb[lo:hi, pair, kb * 128:(kb + 1) * 128],
                qhT_sb[lo:hi, pair, r * RNG + f0:(r + 1) * RNG],
                start=True, stop=True)
            nc.scalar.activation(
                out=probs[:, f0:], in_=sc[:, 0:w],
                func=mybir.ActivationFunctionType.Exp,
                bias=pb_sb[:, kb:kb + 1],
                scale=1.0 / math.sqrt(dh))
            if f0 > 0 or kb * 128 == r * RNG:
                # partial band: keep f - f0 >= p
                nc.gpsimd.affine_select(
                    out=probs[:, f0:f0 + 128],
                    in_=probs[:, f0:f0 + 128],
                    pattern=[[1, 128]],
                    base=0,
                    channel_multiplier=-1,
                    compare_op=mybir.AluOpType.is_ge,
                    fill=0.0)

        def ctx_mm(pair, r, kb, h2, ctx_ps, probs, nch):
            f0 = max(0, kb * 128 - r * RNG)
            h = 2 * pair + h2
            nc.tensor.matmul(
                ctx_ps[:, f0:],
                vh_sb[:, kb, h * (dh + 1):(h + 1) * (dh + 1)],
                probs[:, f0:],
                start=kb == 0, stop=kb == nch - 1)

        for pair in range(cfg.PAIRS):
            for r in range(cfg.NR):
                nch = min(((r + 1) * RNG) // 128, cfg.KB_MAX)
                ctx_ps = [psum.tile([dh + 1, RNG], F32, tag="ctx",
                                    bufs=2, name=f"ctx_ps{h2}")
                          for h2 in range(2)]
                prev = None
                for kb in range(nch):
                    sc = [psum.tile([128, RNG], F32, tag="sc", bufs=3,
                                    name=f"sc{h2}") for h2 in range(2)]
                    probs = [att.tile([128, RNG], BF16, tag=f"pr{h2}",
                                      bufs=3, name=f"probs{h2}")
                             for h2 in range(2)]
                    # software pipeline: ctx of kb-1 interleaves between the
                    # two score matmuls of kb so the PE never waits on exp.
                    score_mm(pair, r, kb, 0, sc[0], probs[0])
                    if prev is not None:
                        ctx_mm(pair, r, kb - 1, 1, ctx_ps[1], prev[1], nch)
                    score_mm(pair, r, kb, 1, sc[1], probs[1])
                    if prev is not None:
                        ctx_mm(pair, r, kb - 1, 0, ctx_ps[0], prev[0], nch)
                    prev = probs
                ctx_mm(pair, r, nch - 1, 0, ctx_ps[0], prev[0], nch)
                ctx_mm(pair, r, nch - 1, 1, ctx_ps[1], prev[1], nch)

                # epilogue: copy psum ctx to SBUF (frees the bank), then
                # divide rows 0..dh-1 by row dh (the prob sum) via a cheap
                # [1,R] reciprocal + partition broadcast + multiply.
                stage = att.tile([128, RNG], BF16, tag="stage", bufs=4)
                for h2 in range(2):
                    csb = small.tile([dh + 1, RNG], F32, tag=f"csb{h2}",
                                     name=f"csb{h2}")
                    nc.vector.tensor_copy(out=csb, in_=ctx_ps[h2])
                    rec = small.tile([1, RNG], F32, tag=f"rec{h2}",
                                     name=f"rec{h2}")
                    nc.vector.reciprocal(rec, csb[dh:dh + 1, :])
                    rbc = small.tile([64, RNG], F32, tag=f"rbc{h2}",
                                     name=f"rbc{h2}")
                    nc.gpsimd.partition_broadcast(rbc, rec)
                    nc.vector.tensor_mul(
                        stage[64 * h2:64 * h2 + 64, :],
                        csb[0:dh, :], rbc)
                # stage rows r*RNG+[0,RNG) as two A2A slots of RSL rows.
                # Staged from the gpsimd queue so the cc0-gated fetch DMAs
                # on the sync queue never block later staging.
                for j in range(2):
                    nc.gpsimd.dma_start(
                        out=a2a_in[pair][2 * r + j, :, :],
                        in_=stage[:, j * RSL:(j + 1) * RSL])
            nc.gpsimd.collective_compute(
                "AllToAll", mybir.AluOpType.bypass,
                replica_groups=[list(range(cfg.NC))],
                ins=[a2a_in[pair][:]], outs=[a2a_out[pair][:]])

        # Scheduler fence (no runtime syncs): without it the scheduler hoists
        # the cc0-gated fetch DMAs ahead of pair-1's staging DMAs on the sync
        # queue, stalling the whole queue until cc0 completes.
        tc.no_sync_barrier()

        # fetch gathered ctx chunks: ccb[(pair, sender)] = sender's 2 heads
        # (128 dims) of pair `pair`, for my RQ rows (RSL per batch half).
        ccb = {}
        for pair in range(cfg.PAIRS):
            for s in range(cfg.NC):
                t_ccb = ctxf.tile([128, RSL], BF16, name=f"ccb_{pair}_{s}",
                                  tag=f"ccb_{pair}_{s}")
                nc.sync.dma_start(out=t_ccb, in_=a2a_out[pair][s, :, :])
                ccb[(pair, s)] = t_ccb

        # ---- P3: Wo + residual + LayerNorm ---------------------------------
        # row-tile t covers my rows [t*128,(t+1)*128): batch b = t//2,
        # in-slot column range (t%2)*128. Sender 4b+s holds head chunk
        # (pair, s) for that batch. Pair-0 chunks are accumulated first so
        # they can proceed while the pair-1 collective is still in flight.
        for t in range(RQ // 128):
            b = t // 2
            col = slice((t % 2) * 128, (t % 2) * 128 + 128)
            pso = [psum.tile([128, cfg.WONW], F32, tag=("pj", "sc")[nsl],
                             bufs=(2, 3)[nsl], name=f"pso{nsl}")
                   for nsl in range(cfg.WON)]
            n_jc = cfg.G * cfg.D4C
            jc = 0
            for pair in range(cfg.PAIRS):
                for s in range(cfg.G):
                    cc = ccb[(pair, 4 * b + s)][:, col]
                    # global output dim chunk for (sender s, pair):
                    oc = s * cfg.D4C + pair
                    for nsl in range(cfg.WON):
                        nc.tensor.matmul(
                            pso[nsl], cc,
                            wo_sb[:, oc, nsl * cfg.WONW:(nsl + 1) * cfg.WONW],
                            start=jc == 0, stop=jc == n_jc - 1)
                    jc += 1
            x = lnp.tile([128, D], F32, tag="x")
            for nsl in range(cfg.WON):
                sl = slice(nsl * cfg.WONW, (nsl + 1) * cfg.WONW)
                nc.vector.tensor_add(x[:, sl], pso[nsl], res_sb[:, t, sl])
            fmax = math.gcd(nc.vector.BN_STATS_FMAX, D)
            nsub = D // fmax
            stats = lnp.tile([128, nsub, nc.vector.BN_STATS_DIM], F32,
                             tag="stats")
            for sg in range(nsub):
                nc.vector.bn_stats(
                    out=stats[:, sg, :],
                    in_=x.rearrange("p (a b) -> p a b", a=nsub)[:, sg, :])
            mv = lnp.tile([128, nc.vector.BN_AGGR_DIM], F32, tag="mv")
            nc.vector.bn_aggr(out=mv, in_=stats)
            sd = lnp.tile([128, 1], F32, tag="sd")
            nc.scalar.activation(out=sd, in_=mv[:, 1:2],
                                 func=mybir.ActivationFunctionType.Sqrt,
                                 bias=eps_sb, scale=1.0)
            rstd = lnp.tile([128, 1], F32, tag="rstd")
            nc.vector.reciprocal(rstd, sd)
            y = lnp.tile([128, D], F32, tag="y")
            nc.vector.tensor_scalar(
                out=y, in0=x, scalar1=mv[:, 0:1], scalar2=rstd,
                op0=mybir.AluOpType.subtract, op1=mybir.AluOpType.mult)
            yg = lnp.tile([128, D], F32, tag="yg")
            nc.vector.tensor_mul(yg, y, gamma_bc)
            out_sb = lnp.tile([128, D], F32, tag="out_sb")
            nc.vector.tensor_add(out_sb, yg, beta_bc)
            nc.sync.dma_start(out=out_shard[t * 128:(t + 1) * 128, :],
                              in_=out_sb)

    nc.compile()
    return nc


def _tile_x(xT, ns_count, nsw, dc=8):
    """[D, S'] -> [ns, 128, dc, nsw] so each n-slice DMA is contiguous."""
    d, s = xT.shape
    cols = ns_count * nsw
    out = xT[:, :cols].reshape(dc, 128, ns_count, nsw)
    return np.ascontiguousarray(out.transpose(2, 1, 0, 3))


def _tile_w(wT):
    """[D, O] -> [128, dc, O] so the weight DMA is contiguous."""
    d, o = wT.shape
    return np.ascontiguousarray(wT.reshape(d // 128, 128, o).transpose(1, 0, 2))


def make_in_maps(cfg: Cfg, q, k, v, Wq, Wk, Wv, Wo, gamma, beta, sen_len):
    """Host-side sharding: slice/transpose/cast/tile per core."""
    bf = ml_dtypes.bfloat16
    in_maps = []
    woT_full = _tile_w(Wo.T.astype(bf))
    pos = np.arange(cfg.S)
    per_batch = {}
    for b in range(cfg.B):
        per_batch[b] = (
            _tile_x(q[b].T.astype(bf), cfg.NS, cfg.NSW),
            _tile_x(k[b].T.astype(bf), cfg.NS_K, cfg.NSW),
            _tile_x(v[b].T.astype(bf), cfg.KB_MAX, 128),
            np.ascontiguousarray(
                np.where(pos < int(sen_len[b]), 0.0, NEG_INF)
                .astype(np.float32).reshape(cfg.KCH, 128).T),
        )
    for c in range(cfg.NC):
        g = c // cfg.G
        l = c % cfg.G
        hs = slice(l * cfg.D4, (l + 1) * cfg.D4)
        rows = slice(c * cfg.RSL, (c + 1) * cfg.RSL)
        qTb, kTb, vTb, pb = per_batch[g]
        res = np.concatenate([q[b, rows, :] for b in range(cfg.B)], axis=0)
        res = res.astype(np.float32).reshape(cfg.G, 128, cfg.D)
        in_maps.append({
            "qT": qTb, "kT": kTb, "vT": vTb,
            "wqT": _tile_w(Wq[hs, :].T.astype(bf)),
            "wkT": _tile_w(Wk[hs, :].T.astype(bf)),
            "wvT": _tile_w(Wv[hs, :].T.astype(bf)),
            "woT": woT_full,
            "resid": np.ascontiguousarray(res.transpose(1, 0, 2)),
            "pad_bias": pb,
            "gamma": gamma.reshape(1, cfg.D).astype(np.float32),
            "beta": beta.reshape(1, cfg.D).astype(np.float32),
        })
    return in_maps


def assemble_output(cfg: Cfg, results):
    out = np.empty((cfg.B, cfg.S, cfg.D), np.float32)
    for c in range(cfg.NC):
        rows = slice(c * cfg.RSL, (c + 1) * cfg.RSL)
        for b in range(cfg.B):
            out[b, rows, :] = results[c]["out_shard"][
                b * cfg.RSL:(b + 1) * cfg.RSL]
    return out


_PROGRAM_CACHE = {}


def _get_program(cfg: Cfg):
    key = (cfg.B, cfg.S, cfg.D, cfg.H, cfg.dh, cfg.KB_MAX)
    if key not in _PROGRAM_CACHE:
        _PROGRAM_CACHE[key] = build_program(cfg)
    return _PROGRAM_CACHE[key]


def run(cfg: Cfg, inputs: dict, trace: bool = False):
    nc = _get_program(cfg)
    in_maps = make_in_maps(cfg, **inputs)
    res = run_bass_kernel_spmd(nc, in_maps, core_ids=list(range(cfg.NC)),
                               trace=trace)
    return assemble_output(cfg, res.results), res


def kernel(**inputs) -> np.ndarray:
    kmax = int(np.max(inputs["sen_len"]))
    cfg = Cfg(B=2, S=2048, D=1024, H=16, dh=64, kmax=kmax)
    out, _ = run(cfg, inputs)
    return out


# revision 24
# speedup vs baseline: 1.4888x; 1.0026x over previous
"""Multi-head attention (projections + causal/padded softmax attention + output
projection + residual + LayerNorm) as a Bass/Tile kernel on 8 Trainium2 cores.

Sharding: tensor-parallel over heads within each batch. Core c handles batch
g = c // 4 and heads [4*(c%4), 4*(c%4)+4). Each core projects Q/K/V for its
4 heads over the full sequence, runs causal attention in a transposed layout
(scoresT[key, row]), and produces ctxT[dh, row]. One 8-way AllToAll per
head-pair redistributes ctxT with a fully STATIC slot map: slot j carries rows
[j*256, (j+1)*256) of the sender's batch, so core j ends up owning that row
range of BOTH batches (cores 0-3 receive batch-0 contributions from cores 0-3
and batch-1 contributions from cores 4-7 in distinct sender slots). No runtime
core-id addressing, no barriers: Tile orders staging DMAs before each
collective and the pair-0 collective overlaps pair-1's attention.

Layout trick: all matmul operands are pre-transposed/pre-cast on the host
(numpy) so every DMA is contiguous: qT/kT/vT = x^T as bf16, WqT/WkT/WvT/WoT =
W^T as bf16. The PE contracts over partitions, so the contraction dim (d_model
or d_head) always sits on the partition axis.

Softmax: scores are bounded (|s| ~ 5) so exp is computed without max
subtraction; exp(scale*s + pad_bias) runs on the scalar engine with the
padding mask folded into the per-key bias. The causal boundary is enforced by
zeroing probs with gpsimd.affine_select. The denominator is obtained by
augmenting V with a ones column (row 64 of ctxT psum = sum of probs).

Attention is software-pipelined for the PE p-state ramp: the ctx matmul of
chunk kb is emitted between the score matmuls of chunk kb+1 so the tensor
engine never waits on the scalar-engine exp. The softmax epilogue first copies
the ctx psum to SBUF (freeing the bank), then does a cheap [1,R] reciprocal,
partition-broadcast and multiply off the critical path.

PSUM budget (8 banks): pj=2 + sc=3 + ctx=2 = 7 (pj/sc shapes are reused for
the Wo accumulators in P3).
"""

import math
from contextlib import ExitStack

import numpy as np
import ml_dtypes

import concourse.bass as bass
import concourse.mybir as mybir
import concourse.tile as tile
from concourse import bacc
from concourse.bass_utils import run_bass_kernel_spmd

BF16 = mybir.dt.bfloat16
F32 = mybir.dt.float32

NEG_INF = -1e9
LN_EPS = 1e-6


class Cfg:
    def __init__(self, B=2, S=2048, D=1024, H=16, dh=64, kmax=None):
        self.B, self.S, self.D, self.H, self.dh = B, S, D, H, dh
        # kmax: max(sen_len) — keys beyond are fully masked, so K/V
        # projection and the attention key loop stop at this bound.
        self.kmax = S if kmax is None else min(int(kmax), S)
        self.NC = 8                      # cores
        self.G = 4                       # cores per batch group
        self.HPC = H // self.G           # heads per core
        self.PAIRS = self.HPC // 2       # head pairs per core
        self.D4 = self.HPC * dh          # per-core projection width
        self.RQ = S // self.G            # rows per core in Wo/LN phase
        self.NR = 4                      # attention row ranges
        self.RNG = S // self.NR          # rows per range (== RQ)
        self.RSL = S // self.NC          # rows per A2A slot (256)
        self.DC = D // 128               # contraction chunks
        self.KCH = S // 128              # key chunks
        self.NS = max(1, S // 512)       # projection n-slices
        self.NSW = S // self.NS          # cols per n-slice
        self.WON = max(1, D // 512)      # Wo n-slices
        self.WONW = D // self.WON
        self.D4C = self.D4 // 128        # 128-chunks in per-core ctx width
        self.KB_MAX = -(-self.kmax // 128)          # key chunks actually used
        self.NS_K = -(-(self.KB_MAX * 128) // self.NSW)  # K-proj n-slices
        assert self.RQ == self.RNG
        assert self.PAIRS >= 1 and self.HPC % 2 == 0


def build_program(cfg: Cfg):
    """Build the (SPMD-identical) Bass program."""
    nc = bacc.Bacc("TRN2", target_bir_lowering=False, debug=False,
                   num_devices=cfg.NC)

    S, D, dh = cfg.S, cfg.D, cfg.dh
    D4, RQ, RNG, RSL = cfg.D4, cfg.RQ, cfg.RNG, cfg.RSL

    # All inputs are pre-tiled on the host so every DMA is contiguous per
    # partition: x inputs as [ns, p, dc, cols], weights as [p, dc, outs].
    qT = nc.dram_tensor("qT", [cfg.NS, 128, cfg.DC, cfg.NSW], BF16,
                        kind="ExternalInput").ap()
    kT = nc.dram_tensor("kT", [cfg.NS_K, 128, cfg.DC, cfg.NSW], BF16,
                        kind="ExternalInput").ap()
    vT = nc.dram_tensor("vT", [cfg.KB_MAX, 128, cfg.DC, 128], BF16,
                        kind="ExternalInput").ap()
    wqT = nc.dram_tensor("wqT", [128, cfg.DC, D4], BF16,
                         kind="ExternalInput").ap()
    wkT = nc.dram_tensor("wkT", [128, cfg.DC, D4], BF16,
                         kind="ExternalInput").ap()
    wvT = nc.dram_tensor("wvT", [128, cfg.DC, D4], BF16,
                         kind="ExternalInput").ap()
    woT = nc.dram_tensor("woT", [128, cfg.DC, D], BF16,
                         kind="ExternalInput").ap()
    resid = nc.dram_tensor("resid", [128, cfg.G, D], F32,
                           kind="ExternalInput").ap()
    pad_bias = nc.dram_tensor("pad_bias", [128, cfg.KCH], F32,
                              kind="ExternalInput").ap()
    gamma = nc.dram_tensor("gamma", [1, D], BF16, kind="ExternalInput").ap()
    beta = nc.dram_tensor("beta", [1, D], F32, kind="ExternalInput").ap()
    out_shard = nc.dram_tensor("out_shard", [RQ, D], F32,
                               kind="ExternalOutput").ap()

    with tile.TileContext(nc) as tc, ExitStack() as ctx:
        consts = ctx.enter_context(tc.tile_pool(name="consts", bufs=1))
        xin = ctx.enter_context(tc.tile_pool(name="xin", bufs=2))
        proj = ctx.enter_context(tc.tile_pool(name="proj", bufs=1))
        att = ctx.enter_context(tc.tile_pool(name="att", bufs=2))
        small = ctx.enter_context(tc.tile_pool(name="small", bufs=2))
        lnp = ctx.enter_context(tc.tile_pool(name="lnp", bufs=2))
        ctxf = ctx.enter_context(tc.tile_pool(name="ctxf", bufs=1))
        dram = ctx.enter_context(
            tc.tile_pool(name="dram", bufs=1, space="DRAM"))
        psum = ctx.enter_context(
            tc.tile_pool(name="psum", bufs=1, space="PSUM"))

        # ---- prologue: all constants (incl. P3's, so P3 never waits) -------
        wq_sb = consts.tile([128, cfg.DC, D4], BF16)
        wk_sb = consts.tile([128, cfg.DC, D4], BF16)
        wv_sb = consts.tile([128, cfg.DC, D4], BF16)
        for w_sb, w_dram in ((wk_sb, wkT), (wv_sb, wvT), (wq_sb, wqT)):
            nc.sync.dma_start(out=w_sb, in_=w_dram)

        pb_sb = consts.tile([128, cfg.KCH], F32)
        nc.sync.dma_start(out=pb_sb, in_=pad_bias)

        # P3 constants (loaded after P1's input stream, see below)
        wo_sb = consts.tile([128, cfg.DC, D], BF16)
        g_row = consts.tile([1, D], BF16)
        b_row = consts.tile([1, D], F32)
        gamma_bc = consts.tile([128, D], BF16)
        beta_bc = consts.tile([128, D], F32)
        eps_sb = consts.tile([128, 1], F32)
        nc.vector.memset(eps_sb, LN_EPS)
        res_sb = consts.tile([128, cfg.G, D], F32)

        # A2A buffers: one per head-pair; slot j = rows [j*RSL,(j+1)*RSL).
        a2a_in = [dram.tile([cfg.NC, 128, RSL], BF16, name=f"a2a_in{p}",
                            tag=f"a2a_in{p}") for p in range(cfg.PAIRS)]
        a2a_out = [dram.tile([cfg.NC, 128, RSL], BF16, name=f"a2a_out{p}",
                             tag=f"a2a_out{p}") for p in range(cfg.PAIRS)]

        # warm up the NRT collective stream during P1: the first collective
        # after the prelude barrier pays ~11us of trigger latency; a dummy
        # 4KB AllToAll absorbs it so cc0/cc1 start promptly.
        warm_in = dram.tile([cfg.NC, 128, 2], BF16, name="warm_in",
                            tag="warm_in")
        warm_out = dram.tile([cfg.NC, 128, 2], BF16, name="warm_out",
                             tag="warm_out")
        nc.gpsimd.collective_compute(
            "AllToAll", mybir.AluOpType.bypass,
            replica_groups=[list(range(cfg.NC))],
            ins=[warm_in[:]], outs=[warm_out[:]])

        # ---- P1: projections (K, V first so attention can start early) ----
        qhT_sb = proj.tile([128, cfg.PAIRS, S], BF16)
        khT_sb = proj.tile([128, cfg.PAIRS, S], BF16)
        vh_sb = proj.tile([128, cfg.KCH, cfg.HPC * (dh + 1)], BF16)

        def qk_proj(x_dram, w_sb, out_sb, ns_count=None):
            for ns in range(ns_count if ns_count is not None else cfg.NS):
                x_ns = xin.tile([128, cfg.DC, cfg.NSW], BF16, tag="x_ns",
                                name="x_ns")
                nc.sync.dma_start(out=x_ns, in_=x_dram[ns])
                for pair in range(cfg.PAIRS):
                    ps = psum.tile([128, cfg.NSW], F32, tag="pj", bufs=2,
                                   name="ps_pj")
                    for dc in range(cfg.DC):
                        nc.tensor.matmul(
                            ps, w_sb[:, dc, pair * 128:(pair + 1) * 128],
                            x_ns[:, dc, :],
                            start=dc == 0, stop=dc == cfg.DC - 1)
                    nc.vector.tensor_copy(
                        out=out_sb[:, pair, ns * cfg.NSW:(ns + 1) * cfg.NSW],
                        in_=ps)

        qk_proj(kT, wk_sb, khT_sb, ns_count=cfg.NS_K)

        for kb in range(cfg.KB_MAX):
            v_kb = xin.tile([128, cfg.DC, 128], BF16, tag="v_kb")
            nc.sync.dma_start(out=v_kb, in_=vT[kb])
            psv = psum.tile([128, D4], F32, tag="pj", bufs=2, name="ps_v")
            for dc in range(cfg.DC):
                nc.tensor.matmul(psv, v_kb[:, dc, :], wv_sb[:, dc, :],
                                 start=dc == 0, stop=dc == cfg.DC - 1)
            nc.vector.tensor_copy(
                out=vh_sb[:, kb, :].rearrange("p (h e) -> p h e", e=dh + 1)
                [:, :, 0:dh],
                in_=psv.rearrange("p (h e) -> p h e", e=dh))
            nc.vector.memset(
                vh_sb[:, kb, :].rearrange("p (h e) -> p h e", e=dh + 1)
                [:, :, dh:dh + 1], 1.0)

        qk_proj(qT, wq_sb, qhT_sb)

        # P3 constants: emitted on the sync queue AFTER P1's input stream so
        # they don't compete for HBM bandwidth before the first matmul; they
        # transfer during P2 and are ready long before P3 needs them.
        nc.sync.dma_start(out=wo_sb, in_=woT)
        nc.sync.dma_start(out=res_sb, in_=resid)
        nc.sync.dma_start(out=g_row, in_=gamma)
        nc.sync.dma_start(out=b_row, in_=beta)
        nc.gpsimd.partition_broadcast(gamma_bc, g_row)
        nc.gpsimd.partition_broadcast(beta_bc, b_row)

        # ---- P2: attention; per-pair A2A overlaps the next pair -----------
        # Both heads' scores go into ONE 2-bank psum tile (cols h2*RNG+...)
        # so a single scalar activation computes exp for both heads.
        def ctx_mm(pair, r, kb, h2, ctx_ps, probs, nch):
            f0 = max(0, kb * 128 - r * RNG)
            h = 2 * pair + h2
            nc.tensor.matmul(
                ctx_ps[:, f0:],
                vh_sb[:, kb, h * (dh + 1):(h + 1) * (dh + 1)],
                probs[:, h2 * RNG + f0:h2 * RNG + RNG],
                start=kb == 0, stop=kb == nch - 1)

        for pair in range(cfg.PAIRS):
            for r in range(cfg.NR):
                nch = min(((r + 1) * RNG) // 128, cfg.KB_MAX)
                ctx_ps = [psum.tile([dh + 1, RNG], F32, tag="ctx",
                                    bufs=2, name=f"ctx_ps{h2}")
                          for h2 in range(2)]
                prev = pp = None
                for kb in range(nch):
                    f0 = max(0, kb * 128 - r * RNG)
                    w = RNG - f0
                    diag = f0 > 0 or kb * 128 == r * RNG
                    sc = psum.tile([128, 2 * RNG], F32, tag="sc", bufs=2,
                                   name="sc")
                    probs = att.tile([128, 2 * RNG], BF16, tag="pr",
                                     bufs=3, name="probs")
                    # software pipeline (depth 2): ctx of kb-2 interleaves
                    # between the score matmuls of kb so the PE never waits
                    # on the (merged, ~1us) exp of kb-1.
                    for h2 in range(2):
                        lo, hi = 64 * h2, 64 * h2 + 64
                        nc.tensor.matmul(
                            sc[:, h2 * RNG:h2 * RNG + w],
                            khT_sb[lo:hi, pair, kb * 128:(kb + 1) * 128],
                            qhT_sb[lo:hi, pair, r * RNG + f0:(r + 1) * RNG],
                            start=True, stop=True)
                        if kb >= 2:
                            ctx_mm(pair, r, kb - 2, h2, ctx_ps[h2],
                                   pp, nch)
                    if w == RNG:
                        nc.scalar.activation(
                            out=probs, in_=sc,
                            func=mybir.ActivationFunctionType.Exp,
                            bias=pb_sb[:, kb:kb + 1],
                            scale=1.0 / math.sqrt(dh))
                    else:
                        for h2 in range(2):
                            nc.scalar.activation(
                                out=probs[:, h2 * RNG + f0:(h2 + 1) * RNG],
                                in_=sc[:, h2 * RNG:h2 * RNG + w],
                                func=mybir.ActivationFunctionType.Exp,
                                bias=pb_sb[:, kb:kb + 1],
                                scale=1.0 / math.sqrt(dh))
                    if diag:
                        # partial band: keep f - f0 >= p
                        for h2 in range(2):
                            nc.gpsimd.affine_select(
                                out=probs[:, h2 * RNG + f0:
                                          h2 * RNG + f0 + 128],
                                in_=probs[:, h2 * RNG + f0:
                                          h2 * RNG + f0 + 128],
                                pattern=[[1, 128]],
                                base=0,
                                channel_multiplier=-1,
                                compare_op=mybir.AluOpType.is_ge,
                                fill=0.0)
                    pp = prev
                    prev = probs
                for kb_t, pr_t in ((nch - 2, pp), (nch - 1, prev)):
                    if kb_t >= 0:
                        for h2 in range(2):
                            ctx_mm(pair, r, kb_t, h2, ctx_ps[h2], pr_t, nch)

                # epilogue: copy psum ctx to SBUF (frees the bank), then
                # divide rows 0..dh-1 by row dh (the prob sum) via a cheap
                # [1,R] reciprocal + partition broadcast + multiply.
                stage = att.tile([128, RNG], BF16, tag="stage", bufs=4)
                for h2 in range(2):
                    csb = small.tile([dh + 1, RNG], F32, tag=f"csb{h2}",
                                     name=f"csb{h2}")
                    nc.vector.tensor_copy(out=csb, in_=ctx_ps[h2])
                    rec = small.tile([1, RNG], F32, tag=f"rec{h2}",
                                     name=f"rec{h2}")
                    nc.vector.reciprocal(rec, csb[dh:dh + 1, :])
                    rbc = small.tile([64, RNG], F32, tag=f"rbc{h2}",
                                     name=f"rbc{h2}")
                    nc.gpsimd.partition_broadcast(rbc, rec)
                    nc.vector.tensor_mul(
                        stage[64 * h2:64 * h2 + 64, :],
                        csb[0:dh, :], rbc)
                # stage rows r*RNG+[0,RNG) as two A2A slots of RSL rows.
                # Staged from the gpsimd queue so the cc0-gated fetch DMAs
                # on the sync queue never block later staging.
                for j in range(2):
                    nc.gpsimd.dma_start(
                        out=a2a_in[pair][2 * r + j, :, :],
                        in_=stage[:, j * RSL:(j + 1) * RSL])
            nc.gpsimd.collective_compute(
                "AllToAll", mybir.AluOpType.bypass,
                replica_groups=[list(range(cfg.NC))],
                ins=[a2a_in[pair][:]], outs=[a2a_out[pair][:]])

        # Scheduler fence (no runtime syncs): without it the scheduler hoists
        # the cc0-gated fetch DMAs ahead of pair-1's staging DMAs on the sync
        # queue, stalling the whole queue until cc0 completes.
        tc.no_sync_barrier()

        # fetch gathered ctx chunks: ccb[(pair, sender)] = sender's 2 heads
        # (128 dims) of pair `pair`, for my RQ rows (RSL per batch half).
        ccb = {}
        for pair in range(cfg.PAIRS):
            for s in range(cfg.NC):
                t_ccb = ctxf.tile([128, RSL], BF16, name=f"ccb_{pair}_{s}",
                                  tag=f"ccb_{pair}_{s}")
                nc.sync.dma_start(out=t_ccb, in_=a2a_out[pair][s, :, :])
                ccb[(pair, s)] = t_ccb

        # ---- P3: Wo + residual + LayerNorm ---------------------------------
        # row-tile t covers my rows [t*128,(t+1)*128): batch b = t//2,
        # in-slot column range (t%2)*128. Sender 4b+s holds head chunk
        # (pair, s) for that batch. Pair-0 chunks are accumulated first so
        # they can proceed while the pair-1 collective is still in flight.
        for t in range(RQ // 128):
            b = t // 2
            col = slice((t % 2) * 128, (t % 2) * 128 + 128)
            pso = [psum.tile([128, cfg.WONW], F32, tag=("pj", "sc")[nsl],
                             bufs=2, name=f"pso{nsl}")
                   for nsl in range(cfg.WON)]
            n_jc = cfg.G * cfg.D4C
            jc = 0
            for pair in range(cfg.PAIRS):
                for s in range(cfg.G):
                    cc = ccb[(pair, 4 * b + s)][:, col]
                    # global output dim chunk for (sender s, pair):
                    oc = s * cfg.D4C + pair
                    for nsl in range(cfg.WON):
                        nc.tensor.matmul(
                            pso[nsl], cc,
                            wo_sb[:, oc, nsl * cfg.WONW:(nsl + 1) * cfg.WONW],
                            start=jc == 0, stop=jc == n_jc - 1)
                    jc += 1
            x = lnp.tile([128, D], F32, tag="x")
            for nsl in range(cfg.WON):
                sl = slice(nsl * cfg.WONW, (nsl + 1) * cfg.WONW)
                nc.vector.tensor_add(x[:, sl], pso[nsl], res_sb[:, t, sl])
            fmax = math.gcd(nc.vector.BN_STATS_FMAX, D)
            nsub = D // fmax
            stats = lnp.tile([128, nsub, nc.vector.BN_STATS_DIM], F32,
                             tag="stats")
            for sg in range(nsub):
                nc.vector.bn_stats(
                    out=stats[:, sg, :],
                    in_=x.rearrange("p (a b) -> p a b", a=nsub)[:, sg, :])
            mv = lnp.tile([128, nc.vector.BN_AGGR_DIM], F32, tag="mv")
            nc.vector.bn_aggr(out=mv, in_=stats)
            sd = lnp.tile([128, 1], F32, tag="sd")
            nc.scalar.activation(out=sd, in_=mv[:, 1:2],
                                 func=mybir.ActivationFunctionType.Sqrt,
                                 bias=eps_sb, scale=1.0)
            rstd = lnp.tile([128, 1], F32, tag="rstd")
            nc.vector.reciprocal(rstd, sd)
            y = lnp.tile([128, D], BF16, tag="y")
            nc.vector.tensor_scalar(
                out=y, in0=x, scalar1=mv[:, 0:1], scalar2=rstd,
                op0=mybir.AluOpType.subtract, op1=mybir.AluOpType.mult)
            yg = lnp.tile([128, D], BF16, tag="yg")
            nc.vector.tensor_mul(yg, y, gamma_bc)
            out_sb = lnp.tile([128, D], F32, tag="out_sb")
            nc.vector.tensor_add(out_sb, yg, beta_bc)
            nc.sync.dma_start(out=out_shard[t * 128:(t + 1) * 128, :],
                              in_=out_sb)

    nc.compile()
    return nc


def _tile_x(xT, ns_count, nsw, dc=8):
    """[D, S'] -> [ns, 128, dc, nsw] so each n-slice DMA is contiguous."""
    d, s = xT.shape
    cols = ns_count * nsw
    out = xT[:, :cols].reshape(dc, 128, ns_count, nsw)
    return np.ascontiguousarray(out.transpose(2, 1, 0, 3))


def _tile_w(wT):
    """[D, O] -> [128, dc, O] so the weight DMA is contiguous."""
    d, o = wT.shape
    return np.ascontiguousarray(wT.reshape(d // 128, 128, o).transpose(1, 0, 2))


def make_in_maps(cfg: Cfg, q, k, v, Wq, Wk, Wv, Wo, gamma, beta, sen_len):
    """Host-side sharding: slice/transpose/cast/tile per core."""
    bf = ml_dtypes.bfloat16
    in_maps = []
    woT_full = _tile_w(Wo.T.astype(bf))
    pos = np.arange(cfg.S)
    per_batch = {}
    for b in range(cfg.B):
        per_batch[b] = (
            _tile_x(q[b].T.astype(bf), cfg.NS, cfg.NSW),
            _tile_x(k[b].T.astype(bf), cfg.NS_K, cfg.NSW),
            _tile_x(v[b].T.astype(bf), cfg.KB_MAX, 128),
            np.ascontiguousarray(
                np.where(pos < int(sen_len[b]), 0.0, NEG_INF)
                .astype(np.float32).reshape(cfg.KCH, 128).T),
        )
    for c in range(cfg.NC):
        g = c // cfg.G
        l = c % cfg.G
        hs = slice(l * cfg.D4, (l + 1) * cfg.D4)
        rows = slice(c * cfg.RSL, (c + 1) * cfg.RSL)
        qTb, kTb, vTb, pb = per_batch[g]
        res = np.concatenate([q[b, rows, :] for b in range(cfg.B)], axis=0)
        res = res.astype(np.float32).reshape(cfg.G, 128, cfg.D)
        in_maps.append({
            "qT": qTb, "kT": kTb, "vT": vTb,
            "wqT": _tile_w(Wq[hs, :].T.astype(bf)),
            "wkT": _tile_w(Wk[hs, :].T.astype(bf)),
            "wvT": _tile_w(Wv[hs, :].T.astype(bf)),
            "woT": woT_full,
            "resid": np.ascontiguousarray(res.transpose(1, 0, 2)),
            "pad_bias": pb,
            "gamma": gamma.reshape(1, cfg.D).astype(bf),
            "beta": beta.reshape(1, cfg.D).astype(np.float32),
        })
    return in_maps


def assemble_output(cfg: Cfg, results):
    out = np.empty((cfg.B, cfg.S, cfg.D), np.float32)
    for c in range(cfg.NC):
        rows = slice(c * cfg.RSL, (c + 1) * cfg.RSL)
        for b in range(cfg.B):
            out[b, rows, :] = results[c]["out_shard"][
                b * cfg.RSL:(b + 1) * cfg.RSL]
    return out


_PROGRAM_CACHE = {}


def _get_program(cfg: Cfg):
    key = (cfg.B, cfg.S, cfg.D, cfg.H, cfg.dh, cfg.KB_MAX)
    if key not in _PROGRAM_CACHE:
        _PROGRAM_CACHE[key] = build_program(cfg)
    return _PROGRAM_CACHE[key]


def run(cfg: Cfg, inputs: dict, trace: bool = False):
    nc = _get_program(cfg)
    in_maps = make_in_maps(cfg, **inputs)
    res = run_bass_kernel_spmd(nc, in_maps, core_ids=list(range(cfg.NC)),
                               trace=trace)
    return assemble_output(cfg, res.results), res


def kernel(**inputs) -> np.ndarray:
    kmax = int(np.max(inputs["sen_len"]))
    cfg = Cfg(B=2, S=2048, D=1024, H=16, dh=64, kmax=kmax)
    out, _ = run(cfg, inputs)
    return out


# revision 27
# speedup vs baseline: 1.5589x; 1.0471x over previous
"""Multi-head attention (projections + causal/padded softmax attention + output
projection + residual + LayerNorm) as a Bass/Tile kernel on 8 Trainium2 cores.

Sharding: tensor-parallel over heads within each batch. Core c handles batch
g = c // 4 and heads [4*(c%4), 4*(c%4)+4). Each core projects Q/K/V for its
4 heads over the full sequence, runs causal attention in a transposed layout
(scoresT[key, row]), and produces ctxT[dh, row]. One 8-way AllToAll per
head-pair redistributes ctxT with a fully STATIC slot map: slot j carries rows
[j*256, (j+1)*256) of the sender's batch, so core j ends up owning that row
range of BOTH batches (cores 0-3 receive batch-0 contributions from cores 0-3
and batch-1 contributions from cores 4-7 in distinct sender slots). No runtime
core-id addressing, no barriers: Tile orders staging DMAs before each
collective and the pair-0 collective overlaps pair-1's attention.

Layout trick: all matmul operands are pre-transposed/pre-cast on the host
(numpy) so every DMA is contiguous: qT/kT/vT = x^T as bf16, WqT/WkT/WvT/WoT =
W^T as bf16. The PE contracts over partitions, so the contraction dim (d_model
or d_head) always sits on the partition axis.

Softmax: scores are bounded (|s| ~ 5) so exp is computed without max
subtraction; exp(scale*s + pad_bias) runs on the scalar engine with the
padding mask folded into the per-key bias. The causal boundary is enforced by
zeroing probs with gpsimd.affine_select. The denominator is obtained by
augmenting V with a ones column (row 64 of ctxT psum = sum of probs).

Attention is software-pipelined for the PE p-state ramp: the ctx matmul of
chunk kb is emitted between the score matmuls of chunk kb+1 so the tensor
engine never waits on the scalar-engine exp. The softmax epilogue first copies
the ctx psum to SBUF (freeing the bank), then does a cheap [1,R] reciprocal,
partition-broadcast and multiply off the critical path.

PSUM budget (8 banks): pj=2 + sc=3 + ctx=2 = 7 (pj/sc shapes are reused for
the Wo accumulators in P3).
"""

import math
from contextlib import ExitStack

import numpy as np
import ml_dtypes

import concourse.bass as bass
import concourse.mybir as mybir
import concourse.tile as tile
from concourse import bacc
from concourse.bass_utils import run_bass_kernel_spmd

BF16 = mybir.dt.bfloat16
F32 = mybir.dt.float32

NEG_INF = -1e9
LN_EPS = 1e-6


class Cfg:
    def __init__(self, B=2, S=2048, D=1024, H=16, dh=64, kmax=None):
        self.B, self.S, self.D, self.H, self.dh = B, S, D, H, dh
        # kmax: max(sen_len) — keys beyond are fully masked, so K/V
        # projection and the attention key loop stop at this bound.
        self.kmax = S if kmax is None else min(int(kmax), S)
        self.NC = 8                      # cores
        self.G = 4                       # cores per batch group
        self.HPC = H // self.G           # heads per core
        self.PAIRS = self.HPC // 2       # head pairs per core
        self.D4 = self.HPC * dh          # per-core projection width
        self.RQ = S // self.G            # rows per core in Wo/LN phase
        self.NR = 4                      # attention row ranges
        self.RNG = S // self.NR          # rows per range (== RQ)
        self.RSL = S // self.NC          # rows per A2A slot (256)
        self.DC = D // 128               # contraction chunks
        self.KCH = S // 128              # key chunks
        self.NS = max(1, S // 512)       # projection n-slices
        self.NSW = S // self.NS          # cols per n-slice
        self.WON = max(1, D // 512)      # Wo n-slices
        self.WONW = D // self.WON
        self.D4C = self.D4 // 128        # 128-chunks in per-core ctx width
        self.KB_MAX = -(-self.kmax // 128)          # key chunks actually used
        self.NS_K = -(-(self.KB_MAX * 128) // self.NSW)  # K-proj n-slices
        assert self.RQ == self.RNG
        assert self.PAIRS >= 1 and self.HPC % 2 == 0


def build_program(cfg: Cfg):
    """Build the (SPMD-identical) Bass program."""
    nc = bacc.Bacc("TRN2", target_bir_lowering=False, debug=False,
                   num_devices=cfg.NC)

    S, D, dh = cfg.S, cfg.D, cfg.dh
    D4, RQ, RNG, RSL = cfg.D4, cfg.RQ, cfg.RNG, cfg.RSL

    # All inputs are pre-tiled on the host so every DMA is contiguous per
    # partition: x inputs as [ns, p, dc, cols], weights as [p, dc, outs].
    qT = nc.dram_tensor("qT", [cfg.NS, 128, cfg.DC, cfg.NSW], BF16,
                        kind="ExternalInput").ap()
    kT = nc.dram_tensor("kT", [cfg.NS_K, 128, cfg.DC, cfg.NSW], BF16,
                        kind="ExternalInput").ap()
    vT = nc.dram_tensor("vT", [cfg.KB_MAX, 128, cfg.DC, 128], BF16,
                        kind="ExternalInput").ap()
    wqT = nc.dram_tensor("wqT", [128, cfg.DC, D4], BF16,
                         kind="ExternalInput").ap()
    wkT = nc.dram_tensor("wkT", [128, cfg.DC, D4], BF16,
                         kind="ExternalInput").ap()
    wvT = nc.dram_tensor("wvT", [128, cfg.DC, D4], BF16,
                         kind="ExternalInput").ap()
    woT = nc.dram_tensor("woT", [128, cfg.DC, D], BF16,
                         kind="ExternalInput").ap()
    resid = nc.dram_tensor("resid", [128, cfg.G, D], F32,
                           kind="ExternalInput").ap()
    pad_bias = nc.dram_tensor("pad_bias", [128, cfg.KCH], F32,
                              kind="ExternalInput").ap()
    gamma = nc.dram_tensor("gamma", [1, D], BF16, kind="ExternalInput").ap()
    beta = nc.dram_tensor("beta", [1, D], F32, kind="ExternalInput").ap()
    out_shard = nc.dram_tensor("out_shard", [RQ, D], F32,
                               kind="ExternalOutput").ap()

    with tile.TileContext(nc) as tc, ExitStack() as ctx:
        consts = ctx.enter_context(tc.tile_pool(name="consts", bufs=1))
        xin = ctx.enter_context(tc.tile_pool(name="xin", bufs=2))
        proj = ctx.enter_context(tc.tile_pool(name="proj", bufs=1))
        att = ctx.enter_context(tc.tile_pool(name="att", bufs=2))
        small = ctx.enter_context(tc.tile_pool(name="small", bufs=2))
        lnp = ctx.enter_context(tc.tile_pool(name="lnp", bufs=2))
        ctxf = ctx.enter_context(tc.tile_pool(name="ctxf", bufs=1))
        dram = ctx.enter_context(
            tc.tile_pool(name="dram", bufs=1, space="DRAM"))
        psum = ctx.enter_context(
            tc.tile_pool(name="psum", bufs=1, space="PSUM"))

        # ---- prologue: all constants (incl. P3's, so P3 never waits) -------
        wq_sb = consts.tile([128, cfg.DC, D4], BF16)
        wk_sb = consts.tile([128, cfg.DC, D4], BF16)
        wv_sb = consts.tile([128, cfg.DC, D4], BF16)
        for w_sb, w_dram in ((wk_sb, wkT), (wv_sb, wvT), (wq_sb, wqT)):
            nc.sync.dma_start(out=w_sb, in_=w_dram)

        pb_sb = consts.tile([128, cfg.KCH], F32)
        nc.sync.dma_start(out=pb_sb, in_=pad_bias)

        # P3 constants (loaded after P1's input stream, see below)
        wo_sb = consts.tile([128, cfg.DC, D], BF16)
        g_row = consts.tile([1, D], BF16)
        b_row = consts.tile([1, D], F32)
        gamma_bc = consts.tile([128, D], BF16)
        beta_bc = consts.tile([128, D], F32)
        eps_sb = consts.tile([128, 1], F32)
        nc.vector.memset(eps_sb, LN_EPS)
        res_sb = consts.tile([128, cfg.G, D], F32)

        # A2A buffers: one per head-pair; slot j = rows [j*RSL,(j+1)*RSL).
        a2a_in = [dram.tile([cfg.NC, 128, RSL], BF16, name=f"a2a_in{p}",
                            tag=f"a2a_in{p}") for p in range(cfg.PAIRS)]
        a2a_out = [dram.tile([cfg.NC, 128, RSL], BF16, name=f"a2a_out{p}",
                             tag=f"a2a_out{p}") for p in range(cfg.PAIRS)]

        # warm up the NRT collective stream during P1: the first collective
        # after the prelude barrier pays ~11us of trigger latency; a dummy
        # 4KB AllToAll absorbs it so cc0/cc1 start promptly.
        warm_in = dram.tile([cfg.NC, 128, 2], BF16, name="warm_in",
                            tag="warm_in")
        warm_out = dram.tile([cfg.NC, 128, 2], BF16, name="warm_out",
                             tag="warm_out")
        nc.gpsimd.collective_compute(
            "AllToAll", mybir.AluOpType.bypass,
            replica_groups=[list(range(cfg.NC))],
            ins=[warm_in[:]], outs=[warm_out[:]])

        # ---- P1: projections (K, V first so attention can start early) ----
        qhT_sb = proj.tile([128, cfg.PAIRS, S], BF16)
        khT_sb = proj.tile([128, cfg.PAIRS, S], BF16)
        vh_sb = proj.tile([128, cfg.KCH, cfg.HPC * (dh + 1)], BF16)

        def qk_proj(x_dram, w_sb, out_sb, ns_count=None):
            for ns in range(ns_count if ns_count is not None else cfg.NS):
                x_ns = xin.tile([128, cfg.DC, cfg.NSW], BF16, tag="x_ns",
                                name="x_ns")
                nc.sync.dma_start(out=x_ns, in_=x_dram[ns])
                for pair in range(cfg.PAIRS):
                    ps = psum.tile([128, cfg.NSW], F32, tag="pj", bufs=2,
                                   name="ps_pj")
                    for dc in range(cfg.DC):
                        nc.tensor.matmul(
                            ps, w_sb[:, dc, pair * 128:(pair + 1) * 128],
                            x_ns[:, dc, :],
                            start=dc == 0, stop=dc == cfg.DC - 1)
                    nc.vector.tensor_copy(
                        out=out_sb[:, pair, ns * cfg.NSW:(ns + 1) * cfg.NSW],
                        in_=ps)

        qk_proj(kT, wk_sb, khT_sb, ns_count=cfg.NS_K)

        for kb in range(cfg.KB_MAX):
            v_kb = xin.tile([128, cfg.DC, 128], BF16, tag="v_kb")
            nc.sync.dma_start(out=v_kb, in_=vT[kb])
            psv = psum.tile([128, D4], F32, tag="pj", bufs=2, name="ps_v")
            for dc in range(cfg.DC):
                nc.tensor.matmul(psv, v_kb[:, dc, :], wv_sb[:, dc, :],
                                 start=dc == 0, stop=dc == cfg.DC - 1)
            nc.vector.tensor_copy(
                out=vh_sb[:, kb, :].rearrange("p (h e) -> p h e", e=dh + 1)
                [:, :, 0:dh],
                in_=psv.rearrange("p (h e) -> p h e", e=dh))
            nc.vector.memset(
                vh_sb[:, kb, :].rearrange("p (h e) -> p h e", e=dh + 1)
                [:, :, dh:dh + 1], 1.0)

        qk_proj(qT, wq_sb, qhT_sb)

        # P3 constants: emitted on the sync queue AFTER P1's input stream so
        # they don't compete for HBM bandwidth before the first matmul; they
        # transfer during P2 and are ready long before P3 needs them.
        nc.sync.dma_start(out=wo_sb, in_=woT)
        nc.sync.dma_start(out=res_sb, in_=resid)
        nc.sync.dma_start(out=g_row, in_=gamma)
        nc.sync.dma_start(out=b_row, in_=beta)
        nc.gpsimd.partition_broadcast(gamma_bc, g_row)
        nc.gpsimd.partition_broadcast(beta_bc, b_row)

        # ---- P2: attention; per-pair A2A overlaps the next pair -----------
        # Both heads' scores go into ONE 2-bank psum tile (cols h2*RNG+...)
        # so a single scalar activation computes exp for both heads.
        def ctx_mm(pair, r, kb, h2, ctx_ps, probs, nch):
            f0 = max(0, kb * 128 - r * RNG)
            h = 2 * pair + h2
            nc.tensor.matmul(
                ctx_ps[:, f0:],
                vh_sb[:, kb, h * (dh + 1):(h + 1) * (dh + 1)],
                probs[:, h2 * RNG + f0:h2 * RNG + RNG],
                start=kb == 0, stop=kb == nch - 1)

        for pair in range(cfg.PAIRS):
            for r in range(cfg.NR):
                nch = min(((r + 1) * RNG) // 128, cfg.KB_MAX)
                ctx_ps = [psum.tile([dh + 1, RNG], F32, tag="ctx",
                                    bufs=2, name=f"ctx_ps{h2}")
                          for h2 in range(2)]
                prev = pp = None
                for kb in range(nch):
                    f0 = max(0, kb * 128 - r * RNG)
                    w = RNG - f0
                    diag = f0 > 0 or kb * 128 == r * RNG
                    sc = psum.tile([128, 2 * RNG], F32, tag="sc", bufs=2,
                                   name="sc")
                    probs = att.tile([128, 2 * RNG], BF16, tag="pr",
                                     bufs=3, name="probs")
                    # software pipeline (depth 2): ctx of kb-2 interleaves
                    # between the score matmuls of kb so the PE never waits
                    # on the (merged, ~1us) exp of kb-1.
                    for h2 in range(2):
                        lo, hi = 64 * h2, 64 * h2 + 64
                        nc.tensor.matmul(
                            sc[:, h2 * RNG:h2 * RNG + w],
                            khT_sb[lo:hi, pair, kb * 128:(kb + 1) * 128],
                            qhT_sb[lo:hi, pair, r * RNG + f0:(r + 1) * RNG],
                            start=True, stop=True)
                        if kb >= 2:
                            ctx_mm(pair, r, kb - 2, h2, ctx_ps[h2],
                                   pp, nch)
                    if w == RNG:
                        nc.scalar.activation(
                            out=probs, in_=sc,
                            func=mybir.ActivationFunctionType.Exp,
                            bias=pb_sb[:, kb:kb + 1],
                            scale=1.0 / math.sqrt(dh))
                    else:
                        for h2 in range(2):
                            nc.scalar.activation(
                                out=probs[:, h2 * RNG + f0:(h2 + 1) * RNG],
                                in_=sc[:, h2 * RNG:h2 * RNG + w],
                                func=mybir.ActivationFunctionType.Exp,
                                bias=pb_sb[:, kb:kb + 1],
                                scale=1.0 / math.sqrt(dh))
                    if diag:
                        # partial band: keep f - f0 >= p
                        for h2 in range(2):
                            nc.gpsimd.affine_select(
                                out=probs[:, h2 * RNG + f0:
                                          h2 * RNG + f0 + 128],
                                in_=probs[:, h2 * RNG + f0:
                                          h2 * RNG + f0 + 128],
                                pattern=[[1, 128]],
                                base=0,
                                channel_multiplier=-1,
                                compare_op=mybir.AluOpType.is_ge,
                                fill=0.0)
                    pp = prev
                    prev = probs
                for kb_t, pr_t in ((nch - 2, pp), (nch - 1, prev)):
                    if kb_t >= 0:
                        for h2 in range(2):
                            ctx_mm(pair, r, kb_t, h2, ctx_ps[h2], pr_t, nch)

                # epilogue: divide rows 0..dh-1 by row dh (the prob sum):
                # bounce the denom row to SBUF, fast-approx reciprocal
                # (the DVE RECIPROCAL op costs a flat ~3.3us!), partition
                # broadcast, then multiply straight out of PSUM.
                stage = att.tile([128, RNG], BF16, tag="stage", bufs=4)
                for h2 in range(2):
                    den = small.tile([1, RNG], F32, tag=f"den{h2}",
                                     name=f"den{h2}")
                    nc.vector.tensor_copy(out=den, in_=ctx_ps[h2][dh:dh + 1])
                    rec = small.tile([1, RNG], F32, tag=f"rec{h2}",
                                     name=f"rec{h2}")
                    nc.vector.reciprocal_approx_fast(rec, den)
                    rbc = small.tile([64, RNG], F32, tag=f"rbc{h2}",
                                     name=f"rbc{h2}")
                    nc.gpsimd.partition_broadcast(rbc, rec)
                    nc.vector.tensor_mul(
                        stage[64 * h2:64 * h2 + 64, :],
                        ctx_ps[h2][0:dh, :], rbc)
                # stage rows r*RNG+[0,RNG) as two A2A slots of RSL rows.
                # Staged from the gpsimd queue so the cc0-gated fetch DMAs
                # on the sync queue never block later staging.
                for j in range(2):
                    nc.gpsimd.dma_start(
                        out=a2a_in[pair][2 * r + j, :, :],
                        in_=stage[:, j * RSL:(j + 1) * RSL])
            nc.gpsimd.collective_compute(
                "AllToAll", mybir.AluOpType.bypass,
                replica_groups=[list(range(cfg.NC))],
                ins=[a2a_in[pair][:]], outs=[a2a_out[pair][:]])

        # Scheduler fence (no runtime syncs): without it the scheduler hoists
        # the cc0-gated fetch DMAs ahead of pair-1's staging DMAs on the sync
        # queue, stalling the whole queue until cc0 completes.
        tc.no_sync_barrier()

        # fetch gathered ctx chunks: ccb[(pair, sender)] = sender's 2 heads
        # (128 dims) of pair `pair`, for my RQ rows (RSL per batch half).
        ccb = {}
        for pair in range(cfg.PAIRS):
            for s in range(cfg.NC):
                t_ccb = ctxf.tile([128, RSL], BF16, name=f"ccb_{pair}_{s}",
                                  tag=f"ccb_{pair}_{s}")
                nc.sync.dma_start(out=t_ccb, in_=a2a_out[pair][s, :, :])
                ccb[(pair, s)] = t_ccb

        # ---- P3: Wo + residual + LayerNorm ---------------------------------
        # row-tile t covers my rows [t*128,(t+1)*128): batch b = t//2,
        # in-slot column range (t%2)*128. Sender 4b+s holds head chunk
        # (pair, s) for that batch. Round A accumulates every pair-0 chunk
        # for ALL row-tiles into SBUF partials while the pair-1 collective
        # is still in flight; round B adds the pair-1 chunks.
        def wo_round(t, pair, pso):
            b = t // 2
            col = slice((t % 2) * 128, (t % 2) * 128 + 128)
            for s in range(cfg.G):
                cc = ccb[(pair, 4 * b + s)][:, col]
                # global output dim chunk for (sender s, pair):
                oc = s * cfg.D4C + pair
                for nsl in range(cfg.WON):
                    nc.tensor.matmul(
                        pso[nsl], cc,
                        wo_sb[:, oc, nsl * cfg.WONW:(nsl + 1) * cfg.WONW],
                        start=s == 0, stop=s == cfg.G - 1)

        partA = []
        for t in range(RQ // 128):
            pso = [psum.tile([128, cfg.WONW], F32, tag=("pj", "sc")[nsl],
                             bufs=2, name=f"psoA{nsl}")
                   for nsl in range(cfg.WON)]
            wo_round(t, 0, pso)
            pa = lnp.tile([128, D], F32, tag="partA", bufs=4)
            for nsl in range(cfg.WON):
                sl = slice(nsl * cfg.WONW, (nsl + 1) * cfg.WONW)
                nc.vector.tensor_add(pa[:, sl], pso[nsl], res_sb[:, t, sl])
            partA.append(pa)

        for t in range(RQ // 128):
            pso = [psum.tile([128, cfg.WONW], F32, tag=("pj", "sc")[nsl],
                             bufs=2, name=f"psoB{nsl}")
                   for nsl in range(cfg.WON)]
            wo_round(t, 1, pso)
            x = lnp.tile([128, D], F32, tag="x")
            for nsl in range(cfg.WON):
                sl = slice(nsl * cfg.WONW, (nsl + 1) * cfg.WONW)
                nc.vector.tensor_add(x[:, sl], pso[nsl], partA[t][:, sl])
            fmax = math.gcd(nc.vector.BN_STATS_FMAX, D)
            nsub = D // fmax
            stats = lnp.tile([128, nsub, nc.vector.BN_STATS_DIM], F32,
                             tag="stats")
            for sg in range(nsub):
                nc.vector.bn_stats(
                    out=stats[:, sg, :],
                    in_=x.rearrange("p (a b) -> p a b", a=nsub)[:, sg, :])
            mv = lnp.tile([128, nc.vector.BN_AGGR_DIM], F32, tag="mv")
            nc.vector.bn_aggr(out=mv, in_=stats)
            sd = lnp.tile([128, 1], F32, tag="sd")
            nc.scalar.activation(out=sd, in_=mv[:, 1:2],
                                 func=mybir.ActivationFunctionType.Sqrt,
                                 bias=eps_sb, scale=1.0)
            rstd = lnp.tile([128, 1], F32, tag="rstd")
            nc.vector.reciprocal_approx_fast(rstd, sd)
            y = lnp.tile([128, D], BF16, tag="y")
            nc.vector.tensor_scalar(
                out=y, in0=x, scalar1=mv[:, 0:1], scalar2=rstd,
                op0=mybir.AluOpType.subtract, op1=mybir.AluOpType.mult)
            yg = lnp.tile([128, D], BF16, tag="yg")
            nc.vector.tensor_mul(yg, y, gamma_bc)
            out_sb = lnp.tile([128, D], F32, tag="out_sb")
            nc.vector.tensor_add(out_sb, yg, beta_bc)
            nc.sync.dma_start(out=out_shard[t * 128:(t + 1) * 128, :],
                              in_=out_sb)

    nc.compile()
    return nc


def _tile_x(xT, ns_count, nsw, dc=8):
    """[D, S'] -> [ns, 128, dc, nsw] so each n-slice DMA is contiguous."""
    d, s = xT.shape
    cols = ns_count * nsw
    out = xT[:, :cols].reshape(dc, 128, ns_count, nsw)
    return np.ascontiguousarray(out.transpose(2, 1, 0, 3))


def _tile_w(wT):
    """[D, O] -> [128, dc, O] so the weight DMA is contiguous."""
    d, o = wT.shape
    return np.ascontiguousarray(wT.reshape(d // 128, 128, o).transpose(1, 0, 2))


def make_in_maps(cfg: Cfg, q, k, v, Wq, Wk, Wv, Wo, gamma, beta, sen_len):
    """Host-side sharding: slice/transpose/cast/tile per core."""
    bf = ml_dtypes.bfloat16
    in_maps = []
    woT_full = _tile_w(Wo.T.astype(bf))
    pos = np.arange(cfg.S)
    per_batch = {}
    for b in range(cfg.B):
        per_batch[b] = (
            _tile_x(q[b].T.astype(bf), cfg.NS, cfg.NSW),
            _tile_x(k[b].T.astype(bf), cfg.NS_K, cfg.NSW),
            _tile_x(v[b].T.astype(bf), cfg.KB_MAX, 128),
            np.ascontiguousarray(
                np.where(pos < int(sen_len[b]), 0.0, NEG_INF)
                .astype(np.float32).reshape(cfg.KCH, 128).T),
        )
    for c in range(cfg.NC):
        g = c // cfg.G
        l = c % cfg.G
        hs = slice(l * cfg.D4, (l + 1) * cfg.D4)
        rows = slice(c * cfg.RSL, (c + 1) * cfg.RSL)
        qTb, kTb, vTb, pb = per_batch[g]
        res = np.concatenate([q[b, rows, :] for b in range(cfg.B)], axis=0)
        res = res.astype(np.float32).reshape(cfg.G, 128, cfg.D)
        in_maps.append({
            "qT": qTb, "kT": kTb, "vT": vTb,
            "wqT": _tile_w(Wq[hs, :].T.astype(bf)),
            "wkT": _tile_w(Wk[hs, :].T.astype(bf)),
            "wvT": _tile_w(Wv[hs, :].T.astype(bf)),
            "woT": woT_full,
            "resid": np.ascontiguousarray(res.transpose(1, 0, 2)),
            "pad_bias": pb,
            "gamma": gamma.reshape(1, cfg.D).astype(bf),
            "beta": beta.reshape(1, cfg.D).astype(np.float32),
        })
    return in_maps


def assemble_output(cfg: Cfg, results):
    out = np.empty((cfg.B, cfg.S, cfg.D), np.float32)
    for c in range(cfg.NC):
        rows = slice(c * cfg.RSL, (c + 1) * cfg.RSL)
        for b in range(cfg.B):
            out[b, rows, :] = results[c]["out_shard"][
                b * cfg.RSL:(b + 1) * cfg.RSL]
    return out


_PROGRAM_CACHE = {}


def _get_program(cfg: Cfg):
    key = (cfg.B, cfg.S, cfg.D, cfg.H, cfg.dh, cfg.KB_MAX)
    if key not in _PROGRAM_CACHE:
        _PROGRAM_CACHE[key] = build_program(cfg)
    return _PROGRAM_CACHE[key]


def run(cfg: Cfg, inputs: dict, trace: bool = False):
    nc = _get_program(cfg)
    in_maps = make_in_maps(cfg, **inputs)
    res = run_bass_kernel_spmd(nc, in_maps, core_ids=list(range(cfg.NC)),
                               trace=trace)
    return assemble_output(cfg, res.results), res


def kernel(**inputs) -> np.ndarray:
    kmax = int(np.max(inputs["sen_len"]))
    cfg = Cfg(B=2, S=2048, D=1024, H=16, dh=64, kmax=kmax)
    out, _ = run(cfg, inputs)
    return out


# revision 30
# speedup vs baseline: 1.5721x; 1.0085x over previous
"""Multi-head attention (projections + causal/padded softmax attention + output
projection + residual + LayerNorm) as a Bass/Tile kernel on 8 Trainium2 cores.

Sharding: tensor-parallel over heads within each batch. Core c handles batch
g = c // 4 and heads [4*(c%4), 4*(c%4)+4). Each core projects Q/K/V for its
4 heads over the full sequence, runs causal attention in a transposed layout
(scoresT[key, row]), and produces ctxT[dh, row]. One 8-way AllToAll per
head-pair redistributes ctxT with a fully STATIC slot map: slot j carries rows
[j*256, (j+1)*256) of the sender's batch, so core j ends up owning that row
range of BOTH batches (cores 0-3 receive batch-0 contributions from cores 0-3
and batch-1 contributions from cores 4-7 in distinct sender slots). No runtime
core-id addressing, no barriers: Tile orders staging DMAs before each
collective and the pair-0 collective overlaps pair-1's attention.

Layout trick: all matmul operands are pre-transposed/pre-cast on the host
(numpy) so every DMA is contiguous: qT/kT/vT = x^T as bf16, WqT/WkT/WvT/WoT =
W^T as bf16. The PE contracts over partitions, so the contraction dim (d_model
or d_head) always sits on the partition axis.

Softmax: scores are bounded (|s| ~ 5) so exp is computed without max
subtraction; exp(scale*s + pad_bias) runs on the scalar engine with the
padding mask folded into the per-key bias. The causal boundary is enforced by
zeroing probs with gpsimd.affine_select. The denominator is obtained by
augmenting V with a ones column (row 64 of ctxT psum = sum of probs).

Attention is software-pipelined for the PE p-state ramp: the ctx matmul of
chunk kb is emitted between the score matmuls of chunk kb+1 so the tensor
engine never waits on the scalar-engine exp. The softmax epilogue first copies
the ctx psum to SBUF (freeing the bank), then does a cheap [1,R] reciprocal,
partition-broadcast and multiply off the critical path.

PSUM budget (8 banks): pj=2 + sc=3 + ctx=2 = 7 (pj/sc shapes are reused for
the Wo accumulators in P3).
"""

import math
from contextlib import ExitStack

import numpy as np
import ml_dtypes

import concourse.bass as bass
import concourse.mybir as mybir
import concourse.tile as tile
from concourse import bacc
from concourse.bass_utils import run_bass_kernel_spmd

BF16 = mybir.dt.bfloat16
F32 = mybir.dt.float32

NEG_INF = -1e9
LN_EPS = 1e-6


class Cfg:
    def __init__(self, B=2, S=2048, D=1024, H=16, dh=64, kmax=None):
        self.B, self.S, self.D, self.H, self.dh = B, S, D, H, dh
        # kmax: max(sen_len) — keys beyond are fully masked, so K/V
        # projection and the attention key loop stop at this bound.
        self.kmax = S if kmax is None else min(int(kmax), S)
        self.NC = 8                      # cores
        self.G = 4                       # cores per batch group
        self.HPC = H // self.G           # heads per core
        self.PAIRS = self.HPC // 2       # head pairs per core
        self.D4 = self.HPC * dh          # per-core projection width
        self.RQ = S // self.G            # rows per core in Wo/LN phase
        self.NR = 4                      # attention row ranges
        self.RNG = S // self.NR          # rows per range (== RQ)
        self.RSL = S // self.NC          # rows per A2A slot (256)
        self.DC = D // 128               # contraction chunks
        self.KCH = S // 128              # key chunks
        self.NS = max(1, S // 512)       # projection n-slices
        self.NSW = S // self.NS          # cols per n-slice
        self.WON = max(1, D // 512)      # Wo n-slices
        self.WONW = D // self.WON
        self.D4C = self.D4 // 128        # 128-chunks in per-core ctx width
        self.KB_MAX = -(-self.kmax // 128)          # key chunks actually used
        self.NS_K = -(-(self.KB_MAX * 128) // self.NSW)  # K-proj n-slices
        assert self.RQ == self.RNG
        assert self.PAIRS >= 1 and self.HPC % 2 == 0


def build_program(cfg: Cfg):
    """Build the (SPMD-identical) Bass program."""
    nc = bacc.Bacc("TRN2", target_bir_lowering=False, debug=False,
                   num_devices=cfg.NC)

    S, D, dh = cfg.S, cfg.D, cfg.dh
    D4, RQ, RNG, RSL = cfg.D4, cfg.RQ, cfg.RNG, cfg.RSL

    # All inputs are pre-tiled on the host so every DMA is contiguous per
    # partition: x inputs as [ns, p, dc, cols], weights as [p, dc, outs].
    qT = nc.dram_tensor("qT", [cfg.NS, 128, cfg.DC, cfg.NSW], BF16,
                        kind="ExternalInput").ap()
    kT = nc.dram_tensor("kT", [cfg.NS_K, 128, cfg.DC, cfg.NSW], BF16,
                        kind="ExternalInput").ap()
    vT = nc.dram_tensor("vT", [cfg.KB_MAX, 128, cfg.DC, 128], BF16,
                        kind="ExternalInput").ap()
    wqT = nc.dram_tensor("wqT", [128, cfg.DC, D4], BF16,
                         kind="ExternalInput").ap()
    wkT = nc.dram_tensor("wkT", [128, cfg.DC, D4], BF16,
                         kind="ExternalInput").ap()
    wvT = nc.dram_tensor("wvT", [128, cfg.DC, D4], BF16,
                         kind="ExternalInput").ap()
    woT = nc.dram_tensor("woT", [128, cfg.DC, D], BF16,
                         kind="ExternalInput").ap()
    resid = nc.dram_tensor("resid", [128, cfg.G, D], F32,
                           kind="ExternalInput").ap()
    pad_bias = nc.dram_tensor("pad_bias", [128, cfg.KCH], F32,
                              kind="ExternalInput").ap()
    gamma = nc.dram_tensor("gamma", [1, D], BF16, kind="ExternalInput").ap()
    beta = nc.dram_tensor("beta", [1, D], F32, kind="ExternalInput").ap()
    out_shard = nc.dram_tensor("out_shard", [RQ, D], F32,
                               kind="ExternalOutput").ap()

    with tile.TileContext(nc) as tc, ExitStack() as ctx:
        consts = ctx.enter_context(tc.tile_pool(name="consts", bufs=1))
        xin = ctx.enter_context(tc.tile_pool(name="xin", bufs=2))
        proj = ctx.enter_context(tc.tile_pool(name="proj", bufs=1))
        att = ctx.enter_context(tc.tile_pool(name="att", bufs=2))
        small = ctx.enter_context(tc.tile_pool(name="small", bufs=2))
        lnp = ctx.enter_context(tc.tile_pool(name="lnp", bufs=2))
        ctxf = ctx.enter_context(tc.tile_pool(name="ctxf", bufs=1))
        dram = ctx.enter_context(
            tc.tile_pool(name="dram", bufs=1, space="DRAM"))
        psum = ctx.enter_context(
            tc.tile_pool(name="psum", bufs=1, space="PSUM"))

        # ---- prologue: all constants (incl. P3's, so P3 never waits) -------
        wq_sb = consts.tile([128, cfg.DC, D4], BF16)
        wk_sb = consts.tile([128, cfg.DC, D4], BF16)
        wv_sb = consts.tile([128, cfg.DC, D4], BF16)
        for w_sb, w_dram in ((wk_sb, wkT), (wv_sb, wvT), (wq_sb, wqT)):
            nc.sync.dma_start(out=w_sb, in_=w_dram)

        pb_sb = consts.tile([128, cfg.KCH], F32)
        nc.sync.dma_start(out=pb_sb, in_=pad_bias)

        # P3 constants (loaded after P1's input stream, see below)
        wo_sb = consts.tile([128, cfg.DC, D], BF16)
        g_row = consts.tile([1, D], BF16)
        b_row = consts.tile([1, D], F32)
        gamma_bc = consts.tile([128, D], BF16)
        beta_bc = consts.tile([128, D], F32)
        eps_sb = consts.tile([128, 1], F32)
        nc.vector.memset(eps_sb, LN_EPS)
        res_sb = consts.tile([128, cfg.G, D], F32)
        # causal triangle bias: tri[p, f] = 0 if f >= p else NEG_INF.
        # Added (by the vector engine) onto the diagonal 128-col band of the
        # scores before exp — keeps the gpsimd queue out of the PE's
        # dependency chain.
        tri = consts.tile([128, 128], F32)
        nc.vector.memset(tri, 0.0)
        nc.gpsimd.affine_select(
            out=tri, in_=tri, pattern=[[1, 128]], base=0,
            channel_multiplier=-1, compare_op=mybir.AluOpType.is_ge,
            fill=NEG_INF)

        # A2A buffers: one per head-pair; slot j = rows [j*RSL,(j+1)*RSL).
        a2a_in = [dram.tile([cfg.NC, 128, RSL], BF16, name=f"a2a_in{p}",
                            tag=f"a2a_in{p}") for p in range(cfg.PAIRS)]
        a2a_out = [dram.tile([cfg.NC, 128, RSL], BF16, name=f"a2a_out{p}",
                             tag=f"a2a_out{p}") for p in range(cfg.PAIRS)]

        # warm up the NRT collective stream during P1: the first collective
        # after the prelude barrier pays ~11us of trigger latency; a dummy
        # 4KB AllToAll absorbs it so cc0/cc1 start promptly.
        warm_in = dram.tile([cfg.NC, 128, 2], BF16, name="warm_in",
                            tag="warm_in")
        warm_out = dram.tile([cfg.NC, 128, 2], BF16, name="warm_out",
                             tag="warm_out")
        nc.gpsimd.collective_compute(
            "AllToAll", mybir.AluOpType.bypass,
            replica_groups=[list(range(cfg.NC))],
            ins=[warm_in[:]], outs=[warm_out[:]])

        # ---- P1: projections (K, V first so attention can start early) ----
        qhT_sb = proj.tile([128, cfg.PAIRS, S], BF16)
        khT_sb = proj.tile([128, cfg.PAIRS, S], BF16)
        vh_sb = proj.tile([128, cfg.KCH, cfg.HPC * (dh + 1)], BF16)

        def qk_proj(x_dram, w_sb, out_sb, ns_count=None):
            for ns in range(ns_count if ns_count is not None else cfg.NS):
                x_ns = xin.tile([128, cfg.DC, cfg.NSW], BF16, tag="x_ns",
                                name="x_ns")
                nc.sync.dma_start(out=x_ns, in_=x_dram[ns])
                for pair in range(cfg.PAIRS):
                    ps = psum.tile([128, cfg.NSW], F32, tag="pj", bufs=2,
                                   name="ps_pj")
                    for dc in range(cfg.DC):
                        nc.tensor.matmul(
                            ps, w_sb[:, dc, pair * 128:(pair + 1) * 128],
                            x_ns[:, dc, :],
                            start=dc == 0, stop=dc == cfg.DC - 1)
                    nc.vector.tensor_copy(
                        out=out_sb[:, pair, ns * cfg.NSW:(ns + 1) * cfg.NSW],
                        in_=ps)

        qk_proj(kT, wk_sb, khT_sb, ns_count=cfg.NS_K)

        for kb in range(cfg.KB_MAX):
            v_kb = xin.tile([128, cfg.DC, 128], BF16, tag="v_kb")
            nc.sync.dma_start(out=v_kb, in_=vT[kb])
            psv = psum.tile([128, D4], F32, tag="pj", bufs=2, name="ps_v")
            for dc in range(cfg.DC):
                nc.tensor.matmul(psv, v_kb[:, dc, :], wv_sb[:, dc, :],
                                 start=dc == 0, stop=dc == cfg.DC - 1)
            nc.vector.tensor_copy(
                out=vh_sb[:, kb, :].rearrange("p (h e) -> p h e", e=dh + 1)
                [:, :, 0:dh],
                in_=psv.rearrange("p (h e) -> p h e", e=dh))
            nc.vector.memset(
                vh_sb[:, kb, :].rearrange("p (h e) -> p h e", e=dh + 1)
                [:, :, dh:dh + 1], 1.0)

        qk_proj(qT, wq_sb, qhT_sb)

        # P3 constants: emitted on the sync queue AFTER P1's input stream so
        # they don't compete for HBM bandwidth before the first matmul; they
        # transfer during P2 and are ready long before P3 needs them.
        nc.sync.dma_start(out=wo_sb, in_=woT)
        nc.sync.dma_start(out=res_sb, in_=resid)
        nc.sync.dma_start(out=g_row, in_=gamma)
        nc.sync.dma_start(out=b_row, in_=beta)
        nc.gpsimd.partition_broadcast(gamma_bc, g_row)
        nc.gpsimd.partition_broadcast(beta_bc, b_row)

        # ---- P2: attention; per-pair A2A overlaps the next pair -----------
        # Both heads' scores go into ONE 2-bank psum tile (cols h2*RNG+...)
        # so a single scalar activation computes exp for both heads.
        def ctx_mm(pair, r, kb, h2, ctx_ps, probs, nch):
            f0 = max(0, kb * 128 - r * RNG)
            h = 2 * pair + h2
            nc.tensor.matmul(
                ctx_ps[:, f0:],
                vh_sb[:, kb, h * (dh + 1):(h + 1) * (dh + 1)],
                probs[:, h2 * RNG + f0:h2 * RNG + RNG],
                start=kb == 0, stop=kb == nch - 1)

        for pair in range(cfg.PAIRS):
            for r in range(cfg.NR):
                nch = min(((r + 1) * RNG) // 128, cfg.KB_MAX)
                ctx_ps = [psum.tile([dh + 1, RNG], F32, tag="ctx",
                                    bufs=2, name=f"ctx_ps{h2}")
                          for h2 in range(2)]
                pend = []  # pending probs tiles awaiting their ctx matmul
                for kb in range(nch):
                    f0 = max(0, kb * 128 - r * RNG)
                    diag = f0 > 0 or kb * 128 == r * RNG
                    sc = psum.tile([128, 2 * RNG], F32, tag="sc", bufs=2,
                                   name="sc")
                    probs = att.tile([128, 2 * RNG], BF16, tag="pr",
                                     bufs=4, name="probs")
                    # software pipeline (depth 3): ctx of kb-3 interleaves
                    # between the score matmuls of kb so the PE never waits
                    # on the tri-add + merged exp of recent chunks.
                    for h2 in range(2):
                        lo, hi = 64 * h2, 64 * h2 + 64
                        nc.tensor.matmul(
                            sc[:, h2 * RNG + f0:(h2 + 1) * RNG],
                            khT_sb[lo:hi, pair, kb * 128:(kb + 1) * 128],
                            qhT_sb[lo:hi, pair, r * RNG + f0:(r + 1) * RNG],
                            start=True, stop=True)
                        if len(pend) == 3:
                            ctx_mm(pair, r, kb - 3, h2, ctx_ps[h2],
                                   pend[0], nch)
                    if len(pend) == 3:
                        pend.pop(0)
                    if diag:
                        # causal boundary: bias the diagonal band before exp
                        for h2 in range(2):
                            band = slice(h2 * RNG + f0, h2 * RNG + f0 + 128)
                            nc.vector.tensor_add(sc[:, band], sc[:, band],
                                                 tri)
                    nc.scalar.activation(
                        out=probs, in_=sc,
                        func=mybir.ActivationFunctionType.Exp,
                        bias=pb_sb[:, kb:kb + 1],
                        scale=1.0 / math.sqrt(dh))
                    pend.append(probs)
                for i, pr_t in enumerate(pend):
                    for h2 in range(2):
                        ctx_mm(pair, r, nch - len(pend) + i, h2,
                               ctx_ps[h2], pr_t, nch)

                # epilogue: divide rows 0..dh-1 by row dh (the prob sum):
                # bounce the denom row to SBUF, fast-approx reciprocal
                # (the DVE RECIPROCAL op costs a flat ~3.3us!), partition
                # broadcast, then multiply straight out of PSUM.
                stage = att.tile([128, RNG], BF16, tag="stage", bufs=4)
                for h2 in range(2):
                    den = small.tile([1, RNG], F32, tag=f"den{h2}",
                                     name=f"den{h2}")
                    nc.vector.tensor_copy(out=den, in_=ctx_ps[h2][dh:dh + 1])
                    rec = small.tile([1, RNG], F32, tag=f"rec{h2}",
                                     name=f"rec{h2}")
                    nc.vector.reciprocal_approx_fast(rec, den)
                    rbc = small.tile([64, RNG], F32, tag=f"rbc{h2}",
                                     name=f"rbc{h2}")
                    nc.gpsimd.partition_broadcast(rbc, rec)
                    nc.vector.tensor_mul(
                        stage[64 * h2:64 * h2 + 64, :],
                        ctx_ps[h2][0:dh, :], rbc)
                # stage rows r*RNG+[0,RNG) as two A2A slots of RSL rows
                for j in range(2):
                    nc.sync.dma_start(
                        out=a2a_in[pair][2 * r + j, :, :],
                        in_=stage[:, j * RSL:(j + 1) * RSL])
            nc.gpsimd.collective_compute(
                "AllToAll", mybir.AluOpType.bypass,
                replica_groups=[list(range(cfg.NC))],
                ins=[a2a_in[pair][:]], outs=[a2a_out[pair][:]])

        # Scheduler fence (no runtime syncs): without it the scheduler hoists
        # the cc0-gated fetch DMAs ahead of pair-1's staging DMAs on the sync
        # queue, stalling the whole queue until cc0 completes.
        tc.no_sync_barrier()

        # fetch gathered ctx chunks: ccb[(pair, sender)] = sender's 2 heads
        # (128 dims) of pair `pair`, for my RQ rows (RSL per batch half).
        ccb = {}
        for pair in range(cfg.PAIRS):
            for s in range(cfg.NC):
                t_ccb = ctxf.tile([128, RSL], BF16, name=f"ccb_{pair}_{s}",
                                  tag=f"ccb_{pair}_{s}")
                nc.sync.dma_start(out=t_ccb, in_=a2a_out[pair][s, :, :])
                ccb[(pair, s)] = t_ccb

        # ---- P3: Wo + residual + LayerNorm ---------------------------------
        # row-tile t covers my rows [t*128,(t+1)*128): batch b = t//2,
        # in-slot column range (t%2)*128. Sender 4b+s holds head chunk
        # (pair, s) for that batch. Round A accumulates every pair-0 chunk
        # for ALL row-tiles into SBUF partials while the pair-1 collective
        # is still in flight; round B adds the pair-1 chunks.
        def wo_round(t, pair, pso):
            b = t // 2
            col = slice((t % 2) * 128, (t % 2) * 128 + 128)
            for s in range(cfg.G):
                cc = ccb[(pair, 4 * b + s)][:, col]
                # global output dim chunk for (sender s, pair):
                oc = s * cfg.D4C + pair
                for nsl in range(cfg.WON):
                    nc.tensor.matmul(
                        pso[nsl], cc,
                        wo_sb[:, oc, nsl * cfg.WONW:(nsl + 1) * cfg.WONW],
                        start=s == 0, stop=s == cfg.G - 1)

        partA = []
        for t in range(RQ // 128):
            pso = [psum.tile([128, cfg.WONW], F32, tag=("pj", "sc")[nsl],
                             bufs=2, name=f"psoA{nsl}")
                   for nsl in range(cfg.WON)]
            wo_round(t, 0, pso)
            pa = lnp.tile([128, D], F32, tag="partA", bufs=4)
            for nsl in range(cfg.WON):
                sl = slice(nsl * cfg.WONW, (nsl + 1) * cfg.WONW)
                nc.vector.tensor_add(pa[:, sl], pso[nsl], res_sb[:, t, sl])
            partA.append(pa)

        for t in range(RQ // 128):
            pso = [psum.tile([128, cfg.WONW], F32, tag=("pj", "sc")[nsl],
                             bufs=2, name=f"psoB{nsl}")
                   for nsl in range(cfg.WON)]
            wo_round(t, 1, pso)
            x = lnp.tile([128, D], F32, tag="x")
            for nsl in range(cfg.WON):
                sl = slice(nsl * cfg.WONW, (nsl + 1) * cfg.WONW)
                nc.vector.tensor_add(x[:, sl], pso[nsl], partA[t][:, sl])
            fmax = math.gcd(nc.vector.BN_STATS_FMAX, D)
            nsub = D // fmax
            stats = lnp.tile([128, nsub, nc.vector.BN_STATS_DIM], F32,
                             tag="stats")
            for sg in range(nsub):
                nc.vector.bn_stats(
                    out=stats[:, sg, :],
                    in_=x.rearrange("p (a b) -> p a b", a=nsub)[:, sg, :])
            mv = lnp.tile([128, nc.vector.BN_AGGR_DIM], F32, tag="mv")
            nc.vector.bn_aggr(out=mv, in_=stats)
            sd = lnp.tile([128, 1], F32, tag="sd")
            nc.scalar.activation(out=sd, in_=mv[:, 1:2],
                                 func=mybir.ActivationFunctionType.Sqrt,
                                 bias=eps_sb, scale=1.0)
            rstd = lnp.tile([128, 1], F32, tag="rstd")
            nc.vector.reciprocal_approx_fast(rstd, sd)
            y = lnp.tile([128, D], BF16, tag="y")
            nc.vector.tensor_scalar(
                out=y, in0=x, scalar1=mv[:, 0:1], scalar2=rstd,
                op0=mybir.AluOpType.subtract, op1=mybir.AluOpType.mult)
            yg = lnp.tile([128, D], BF16, tag="yg")
            nc.vector.tensor_mul(yg, y, gamma_bc)
            out_sb = lnp.tile([128, D], F32, tag="out_sb")
            nc.vector.tensor_add(out_sb, yg, beta_bc)
            nc.sync.dma_start(out=out_shard[t * 128:(t + 1) * 128, :],
                              in_=out_sb)

    nc.compile()
    return nc


def _tile_x(xT, ns_count, nsw, dc=8):
    """[D, S'] -> [ns, 128, dc, nsw] so each n-slice DMA is contiguous."""
    d, s = xT.shape
    cols = ns_count * nsw
    out = xT[:, :cols].reshape(dc, 128, ns_count, nsw)
    return np.ascontiguousarray(out.transpose(2, 1, 0, 3))


def _tile_w(wT):
    """[D, O] -> [128, dc, O] so the weight DMA is contiguous."""
    d, o = wT.shape
    return np.ascontiguousarray(wT.reshape(d // 128, 128, o).transpose(1, 0, 2))


def make_in_maps(cfg: Cfg, q, k, v, Wq, Wk, Wv, Wo, gamma, beta, sen_len):
    """Host-side sharding: slice/transpose/cast/tile per core."""
    bf = ml_dtypes.bfloat16
    in_maps = []
    woT_full = _tile_w(Wo.T.astype(bf))
    pos = np.arange(cfg.S)
    per_batch = {}
    for b in range(cfg.B):
        per_batch[b] = (
            _tile_x(q[b].T.astype(bf), cfg.NS, cfg.NSW),
            _tile_x(k[b].T.astype(bf), cfg.NS_K, cfg.NSW),
            _tile_x(v[b].T.astype(bf), cfg.KB_MAX, 128),
            np.ascontiguousarray(
                np.where(pos < int(sen_len[b]), 0.0, NEG_INF)
                .astype(np.float32).reshape(cfg.KCH, 128).T),
        )
    for c in range(cfg.NC):
        g = c // cfg.G
        l = c % cfg.G
        hs = slice(l * cfg.D4, (l + 1) * cfg.D4)
        rows = slice(c * cfg.RSL, (c + 1) * cfg.RSL)
        qTb, kTb, vTb, pb = per_batch[g]
        res = np.concatenate([q[b, rows, :] for b in range(cfg.B)], axis=0)
        res = res.astype(np.float32).reshape(cfg.G, 128, cfg.D)
        in_maps.append({
            "qT": qTb, "kT": kTb, "vT": vTb,
            "wqT": _tile_w(Wq[hs, :].T.astype(bf)),
            "wkT": _tile_w(Wk[hs, :].T.astype(bf)),
            "wvT": _tile_w(Wv[hs, :].T.astype(bf)),
            "woT": woT_full,
            "resid": np.ascontiguousarray(res.transpose(1, 0, 2)),
            "pad_bias": pb,
            "gamma": gamma.reshape(1, cfg.D).astype(bf),
            "beta": beta.reshape(1, cfg.D).astype(np.float32),
        })
    return in_maps


def assemble_output(cfg: Cfg, results):
    out = np.empty((cfg.B, cfg.S, cfg.D), np.float32)
    for c in range(cfg.NC):
        rows = slice(c * cfg.RSL, (c + 1) * cfg.RSL)
        for b in range(cfg.B):
            out[b, rows, :] = results[c]["out_shard"][
                b * cfg.RSL:(b + 1) * cfg.RSL]
    return out


_PROGRAM_CACHE = {}


def _get_program(cfg: Cfg):
    key = (cfg.B, cfg.S, cfg.D, cfg.H, cfg.dh, cfg.KB_MAX)
    if key not in _PROGRAM_CACHE:
        _PROGRAM_CACHE[key] = build_program(cfg)
    return _PROGRAM_CACHE[key]


def run(cfg: Cfg, inputs: dict, trace: bool = False):
    nc = _get_program(cfg)
    in_maps = make_in_maps(cfg, **inputs)
    res = run_bass_kernel_spmd(nc, in_maps, core_ids=list(range(cfg.NC)),
                               trace=trace)
    return assemble_output(cfg, res.results), res


def kernel(**inputs) -> np.ndarray:
    kmax = int(np.max(inputs["sen_len"]))
    cfg = Cfg(B=2, S=2048, D=1024, H=16, dh=64, kmax=kmax)
    out, _ = run(cfg, inputs)
    return out


# revision 38
# speedup vs baseline: 1.8902x; 1.2023x over previous
"""Multi-head attention (projections + causal/padded softmax attention + output
projection + residual + LayerNorm) as a Bass/Tile kernel on 8 Trainium2 cores.

Sharding: tensor-parallel over heads within each batch. Core c handles batch
g = c // 4 and heads [4*(c%4), 4*(c%4)+4). Each core projects Q/K/V for its
4 heads over the full sequence, runs causal attention in a transposed layout
(scoresT[key, row]), and produces ctxT[dh, row]. One 8-way AllToAll per
head-pair redistributes ctxT with a fully STATIC slot map: slot j carries rows
[j*256, (j+1)*256) of the sender's batch, so core j ends up owning that row
range of BOTH batches (cores 0-3 receive batch-0 contributions from cores 0-3
and batch-1 contributions from cores 4-7 in distinct sender slots). No runtime
core-id addressing, no barriers: Tile orders staging DMAs before each
collective and the pair-0 collective overlaps pair-1's attention.

Layout trick: all matmul operands are pre-transposed/pre-cast on the host
(numpy) so every DMA is contiguous: qT/kT/vT = x^T as bf16, WqT/WkT/WvT/WoT =
W^T as bf16. The PE contracts over partitions, so the contraction dim (d_model
or d_head) always sits on the partition axis.

Softmax: scores are bounded (|s| ~ 5) so exp is computed without max
subtraction; exp(scale*s + pad_bias) runs on the scalar engine with the
padding mask folded into the per-key bias. The causal boundary is enforced by
zeroing probs with gpsimd.affine_select. The denominator is obtained by
augmenting V with a ones column (row 64 of ctxT psum = sum of probs).

Attention is software-pipelined for the PE p-state ramp: the ctx matmul of
chunk kb is emitted between the score matmuls of chunk kb+1 so the tensor
engine never waits on the scalar-engine exp. The softmax epilogue first copies
the ctx psum to SBUF (freeing the bank), then does a cheap [1,R] reciprocal,
partition-broadcast and multiply off the critical path.

PSUM budget (8 banks): pj=2 + sc=3 + ctx=2 = 7 (pj/sc shapes are reused for
the Wo accumulators in P3).
"""

import math
from contextlib import ExitStack

import numpy as np
import ml_dtypes

import concourse.bass as bass
import concourse.mybir as mybir
import concourse.tile as tile
from concourse import bacc
from concourse.bass_utils import run_bass_kernel_spmd

BF16 = mybir.dt.bfloat16
F32 = mybir.dt.float32

NEG_INF = -1e9
LN_EPS = 1e-6


class Cfg:
    def __init__(self, B=2, S=2048, D=1024, H=16, dh=64, kmax=None):
        self.B, self.S, self.D, self.H, self.dh = B, S, D, H, dh
        # kmax: max(sen_len) — keys beyond are fully masked, so K/V
        # projection and the attention key loop stop at this bound.
        self.kmax = S if kmax is None else min(int(kmax), S)
        self.NC = 8                      # cores
        self.G = 4                       # cores per batch group
        self.HPC = H // self.G           # heads per core
        self.PAIRS = self.HPC // 2       # head pairs per core
        self.D4 = self.HPC * dh          # per-core projection width
        self.RQ = S // self.G            # rows per core in Wo/LN phase
        self.NR = 4                      # attention row ranges
        self.RNG = S // self.NR          # rows per range (== RQ)
        self.RSL = S // self.NC          # rows per A2A slot (256)
        self.DC = D // 128               # contraction chunks
        self.KCH = S // 128              # key chunks
        self.NS = max(1, S // 512)       # projection n-slices
        self.NSW = S // self.NS          # cols per n-slice
        self.WON = max(1, D // 512)      # Wo n-slices
        self.WONW = D // self.WON
        self.D4C = self.D4 // 128        # 128-chunks in per-core ctx width
        self.KB_MAX = -(-self.kmax // 128)          # key chunks actually used
        self.NS_K = -(-(self.KB_MAX * 128) // self.NSW)  # K-proj n-slices
        assert self.RQ == self.RNG
        assert self.PAIRS >= 1 and self.HPC % 2 == 0


def build_program(cfg: Cfg):
    """Build the (SPMD-identical) Bass program."""
    nc = bacc.Bacc("TRN2", target_bir_lowering=False, debug=False,
                   num_devices=cfg.NC)

    S, D, dh = cfg.S, cfg.D, cfg.dh
    D4, RQ, RNG, RSL = cfg.D4, cfg.RQ, cfg.RNG, cfg.RSL

    # All inputs are pre-tiled on the host so every DMA is contiguous per
    # partition: x inputs as [ns, p, dc, cols], weights as [p, dc, outs].
    qT = nc.dram_tensor("qT", [cfg.NS, 128, cfg.DC, cfg.NSW], BF16,
                        kind="ExternalInput").ap()
    kT = nc.dram_tensor("kT", [cfg.NS_K, 128, cfg.DC, cfg.NSW], BF16,
                        kind="ExternalInput").ap()
    vT = nc.dram_tensor("vT", [cfg.KB_MAX, 128, cfg.DC, 128], BF16,
                        kind="ExternalInput").ap()
    wqT = nc.dram_tensor("wqT", [128, cfg.DC, D4], BF16,
                         kind="ExternalInput").ap()
    wkT = nc.dram_tensor("wkT", [128, cfg.DC, D4], BF16,
                         kind="ExternalInput").ap()
    wvT = nc.dram_tensor("wvT", [128, cfg.DC, D4], BF16,
                         kind="ExternalInput").ap()
    woT = nc.dram_tensor("woT", [128, cfg.DC, D], BF16,
                         kind="ExternalInput").ap()
    resid = nc.dram_tensor("resid", [128, cfg.G, D], F32,
                           kind="ExternalInput").ap()
    pad_bias = nc.dram_tensor("pad_bias", [128, cfg.KCH], F32,
                              kind="ExternalInput").ap()
    gamma = nc.dram_tensor("gamma", [1, D], BF16, kind="ExternalInput").ap()
    beta = nc.dram_tensor("beta", [1, D], F32, kind="ExternalInput").ap()
    out_shard = nc.dram_tensor("out_shard", [RQ, D], F32,
                               kind="ExternalOutput").ap()

    with tile.TileContext(nc) as tc, ExitStack() as ctx:
        consts = ctx.enter_context(tc.tile_pool(name="consts", bufs=1))
        xin = ctx.enter_context(tc.tile_pool(name="xin", bufs=2))
        proj = ctx.enter_context(tc.tile_pool(name="proj", bufs=1))
        att = ctx.enter_context(tc.tile_pool(name="att", bufs=2))
        small = ctx.enter_context(tc.tile_pool(name="small", bufs=2))
        lnp = ctx.enter_context(tc.tile_pool(name="lnp", bufs=2))
        ctxf = ctx.enter_context(tc.tile_pool(name="ctxf", bufs=1))
        dram = ctx.enter_context(
            tc.tile_pool(name="dram", bufs=1, space="DRAM"))
        psum = ctx.enter_context(
            tc.tile_pool(name="psum", bufs=1, space="PSUM"))

        # ---- prologue: all constants (incl. P3's, so P3 never waits) -------
        wq_sb = consts.tile([128, cfg.DC, D4], BF16)
        wk_sb = consts.tile([128, cfg.DC, D4], BF16)
        wv_sb = consts.tile([128, cfg.DC, D4], BF16)
        for w_sb, w_dram in ((wk_sb, wkT), (wv_sb, wvT), (wq_sb, wqT)):
            nc.sync.dma_start(out=w_sb, in_=w_dram)

        pb_sb = consts.tile([128, cfg.KCH], F32)
        nc.sync.dma_start(out=pb_sb, in_=pad_bias)

        # P3 constants (loaded after P1's input stream, see below)
        wo_sb = consts.tile([128, cfg.DC, D], BF16)
        g_row = consts.tile([1, D], BF16)
        b_row = consts.tile([1, D], F32)
        gamma_bc = consts.tile([128, D], BF16)
        beta_bc = consts.tile([128, D], F32)
        eps_sb = consts.tile([128, 1], F32)
        nc.vector.memset(eps_sb, LN_EPS)
        res_sb = consts.tile([128, cfg.G, D], F32)
        # causal triangle bias: tri[p, f] = 0 if f >= p else NEG_INF.
        # Added (by the vector engine) onto the diagonal 128-col band of the
        # scores before exp — keeps the gpsimd queue out of the PE's
        # dependency chain.
        tri = consts.tile([128, 128], F32)
        nc.vector.memset(tri, 0.0)
        nc.gpsimd.affine_select(
            out=tri, in_=tri, pattern=[[1, 128]], base=0,
            channel_multiplier=-1, compare_op=mybir.AluOpType.is_ge,
            fill=NEG_INF)

        # A2A buffers: one per head-pair; slot j = rows [j*RSL,(j+1)*RSL).
        a2a_in = [dram.tile([cfg.NC, 128, RSL], BF16, name=f"a2a_in{p}",
                            tag=f"a2a_in{p}") for p in range(cfg.PAIRS)]
        a2a_out = [dram.tile([cfg.NC, 128, RSL], BF16, name=f"a2a_out{p}",
                             tag=f"a2a_out{p}") for p in range(cfg.PAIRS)]

        # warm up the NRT collective stream during P1: the first collective
        # after the prelude barrier pays ~11us of trigger latency; a dummy
        # 4KB AllToAll absorbs it so cc0/cc1 start promptly.
        warm_in = dram.tile([cfg.NC, 128, 2], BF16, name="warm_in",
                            tag="warm_in")
        warm_out = dram.tile([cfg.NC, 128, 2], BF16, name="warm_out",
                             tag="warm_out")
        nc.gpsimd.collective_compute(
            "AllToAll", mybir.AluOpType.bypass,
            replica_groups=[list(range(cfg.NC))],
            ins=[warm_in[:]], outs=[warm_out[:]])

        # ---- P1: projections (K, V first so attention can start early) ----
        # K/V SBUF is sized to the kmax actually used, not full S.
        qhT_sb = proj.tile([128, cfg.PAIRS, S], BF16)
        khT_sb = proj.tile([128, cfg.PAIRS, cfg.NS_K * cfg.NSW], BF16)
        vh_sb = proj.tile([128, cfg.KB_MAX, cfg.HPC * (dh + 1)], BF16)

        def qk_proj(x_dram, w_sb, out_sb, ns_count=None):
            for ns in range(ns_count if ns_count is not None else cfg.NS):
                x_ns = xin.tile([128, cfg.DC, cfg.NSW], BF16, tag="x_ns",
                                name="x_ns")
                nc.sync.dma_start(out=x_ns, in_=x_dram[ns])
                for pair in range(cfg.PAIRS):
                    ps = psum.tile([128, cfg.NSW], F32, tag="sc", bufs=3,
                                   name="ps_pj")
                    for dc in range(cfg.DC):
                        nc.tensor.matmul(
                            ps, w_sb[:, dc, pair * 128:(pair + 1) * 128],
                            x_ns[:, dc, :],
                            start=dc == 0, stop=dc == cfg.DC - 1)
                    nc.vector.tensor_copy(
                        out=out_sb[:, pair, ns * cfg.NSW:(ns + 1) * cfg.NSW],
                        in_=ps)

        qk_proj(kT, wk_sb, khT_sb, ns_count=cfg.NS_K)

        for kb in range(cfg.KB_MAX):
            v_kb = xin.tile([128, cfg.DC, 128], BF16, tag="v_kb")
            nc.sync.dma_start(out=v_kb, in_=vT[kb])
            psv = psum.tile([128, D4], F32, tag="sc", bufs=3, name="ps_v")
            for dc in range(cfg.DC):
                nc.tensor.matmul(psv, v_kb[:, dc, :], wv_sb[:, dc, :],
                                 start=dc == 0, stop=dc == cfg.DC - 1)
            nc.vector.tensor_copy(
                out=vh_sb[:, kb, :].rearrange("p (h e) -> p h e", e=dh + 1)
                [:, :, 0:dh],
                in_=psv.rearrange("p (h e) -> p h e", e=dh))
            nc.vector.memset(
                vh_sb[:, kb, :].rearrange("p (h e) -> p h e", e=dh + 1)
                [:, :, dh:dh + 1], 1.0)

        # Q: slice 0 is projected in P1 (attention r0 needs it); slices 1..3
        # are interleaved into pair-0's attention as scalar-independent PE
        # filler so the tensor engine never idles waiting on exp (idling
        # resets the PE p-state ramp and halves its clock).
        qx = []
        for ns in range(cfg.NS):
            x_q = xin.tile([128, cfg.DC, cfg.NSW], BF16, tag="qx", bufs=4,
                           name="x_q")
            nc.sync.dma_start(out=x_q, in_=qT[ns])
            qx.append(x_q)

        def q_slice_mm(ns, pair, dc, qp):
            nc.tensor.matmul(
                qp, wq_sb[:, dc, pair * 128:(pair + 1) * 128],
                qx[ns][:, dc, :], start=dc == 0, stop=dc == cfg.DC - 1)
            if dc == cfg.DC - 1:
                nc.vector.tensor_copy(
                    out=qhT_sb[:, pair, ns * cfg.NSW:(ns + 1) * cfg.NSW],
                    in_=qp)

        class QSliceJob:
            def __init__(self, ns):
                self.ns = ns
                self.items = [(p, dc) for p in range(cfg.PAIRS)
                              for dc in range(cfg.DC)]
                self.idx = 0
                self.qp = None

            def emit(self, n):
                for _ in range(n):
                    if self.idx >= len(self.items):
                        return
                    p, dc = self.items[self.idx]
                    self.idx += 1
                    if dc == 0:
                        self.qp = psum.tile([128, cfg.NSW], F32, tag="sc",
                                            bufs=3, name="qp")
                    q_slice_mm(self.ns, p, dc, self.qp)

            def finish(self):
                self.emit(len(self.items) - self.idx)

        qs0 = QSliceJob(0)
        qs0.finish()

        # P3 constants: emitted on the sync queue AFTER P1's input stream so
        # they don't compete for HBM bandwidth before the first matmul; they
        # transfer during P2 and are ready long before P3 needs them.
        nc.sync.dma_start(out=wo_sb, in_=woT)
        nc.sync.dma_start(out=res_sb, in_=resid)
        nc.sync.dma_start(out=g_row, in_=gamma)
        nc.sync.dma_start(out=b_row, in_=beta)
        nc.gpsimd.partition_broadcast(gamma_bc, g_row)
        nc.gpsimd.partition_broadcast(beta_bc, b_row)

        # ---- P2: attention; per-pair A2A overlaps the next pair -----------
        # Both heads' scores go into ONE 2-bank psum tile (cols h2*RNG+...)
        # so a single scalar activation computes exp for both heads.
        def ctx_mm(pair, r, kb, h2, ctx_ps, probs, nch):
            f0 = max(0, kb * 128 - r * RNG)
            h = 2 * pair + h2
            nc.tensor.matmul(
                ctx_ps[:, f0:],
                vh_sb[:, kb, h * (dh + 1):(h + 1) * (dh + 1)],
                probs[:, h2 * RNG + f0:h2 * RNG + RNG],
                start=kb == 0, stop=kb == nch - 1)

        for pair in range(cfg.PAIRS):
            for r in range(cfg.NR):
                nch = min(((r + 1) * RNG) // 128, cfg.KB_MAX)
                qsj = None
                if pair == 0 and r + 1 < cfg.NS:
                    qsj = QSliceJob(r + 1)
                    per_kb = -(-len(qsj.items) // nch)
                ctx_ps = [psum.tile([dh + 1, RNG], F32, tag="ctx",
                                    bufs=2, name=f"ctx_ps{h2}")
                          for h2 in range(2)]
                pend = []  # pending probs tiles awaiting their ctx matmul
                for kb in range(nch):
                    f0 = max(0, kb * 128 - r * RNG)
                    diag = f0 > 0 or kb * 128 == r * RNG
                    sc = psum.tile([128, 2 * RNG], F32, tag="sc", bufs=3,
                                   name="sc")
                    probs = att.tile([128, 2 * RNG], BF16, tag="pr",
                                     bufs=4, name="probs")
                    # software pipeline (depth 3): ctx of kb-3 interleaves
                    # between the score matmuls of kb so the PE never waits
                    # on the tri-add + merged exp of recent chunks.
                    for h2 in range(2):
                        lo, hi = 64 * h2, 64 * h2 + 64
                        nc.tensor.matmul(
                            sc[:, h2 * RNG + f0:(h2 + 1) * RNG],
                            khT_sb[lo:hi, pair, kb * 128:(kb + 1) * 128],
                            qhT_sb[lo:hi, pair, r * RNG + f0:(r + 1) * RNG],
                            start=True, stop=True)
                        if len(pend) == 3:
                            ctx_mm(pair, r, kb - 3, h2, ctx_ps[h2],
                                   pend[0], nch)
                    if len(pend) == 3:
                        pend.pop(0)
                    if diag:
                        # causal boundary: bias the diagonal band before exp
                        for h2 in range(2):
                            band = slice(h2 * RNG + f0, h2 * RNG + f0 + 128)
                            nc.vector.tensor_add(sc[:, band], sc[:, band],
                                                 tri)
                    nc.scalar.activation(
                        out=probs, in_=sc,
                        func=mybir.ActivationFunctionType.Exp,
                        bias=pb_sb[:, kb:kb + 1],
                        scale=1.0 / math.sqrt(dh))
                    if qsj is not None:
                        qsj.emit(per_kb)
                    pend.append(probs)
                if qsj is not None:
                    qsj.finish()
                for i, pr_t in enumerate(pend):
                    for h2 in range(2):
                        ctx_mm(pair, r, nch - len(pend) + i, h2,
                               ctx_ps[h2], pr_t, nch)

                # epilogue: divide rows 0..dh-1 by row dh (the prob sum):
                # bounce the denom row to SBUF, fast-approx reciprocal
                # (the DVE RECIPROCAL op costs a flat ~3.3us!), partition
                # broadcast, then multiply straight out of PSUM.
                stage = att.tile([128, RNG], BF16, tag="stage", bufs=4)
                for h2 in range(2):
                    den = small.tile([1, RNG], F32, tag=f"den{h2}",
                                     name=f"den{h2}")
                    nc.vector.tensor_copy(out=den, in_=ctx_ps[h2][dh:dh + 1])
                    rec = small.tile([1, RNG], F32, tag=f"rec{h2}",
                                     name=f"rec{h2}")
                    nc.vector.reciprocal_approx_fast(rec, den)
                    rbc = small.tile([64, RNG], F32, tag=f"rbc{h2}",
                                     name=f"rbc{h2}")
                    nc.gpsimd.partition_broadcast(rbc, rec)
                    nc.vector.tensor_mul(
                        stage[64 * h2:64 * h2 + 64, :],
                        ctx_ps[h2][0:dh, :], rbc)
                # stage rows r*RNG+[0,RNG) as two A2A slots of RSL rows
                for j in range(2):
                    nc.sync.dma_start(
                        out=a2a_in[pair][2 * r + j, :, :],
                        in_=stage[:, j * RSL:(j + 1) * RSL])
            nc.gpsimd.collective_compute(
                "AllToAll", mybir.AluOpType.bypass,
                replica_groups=[list(range(cfg.NC))],
                ins=[a2a_in[pair][:]], outs=[a2a_out[pair][:]])

        # Scheduler fence (no runtime syncs): without it the scheduler hoists
        # the cc0-gated fetch DMAs ahead of pair-1's staging DMAs on the sync
        # queue, stalling the whole queue until cc0 completes.
        tc.no_sync_barrier()

        # fetch gathered ctx chunks: ccb[(pair, sender)] = sender's 2 heads
        # (128 dims) of pair `pair`, for my RQ rows (RSL per batch half).
        ccb = {}
        for pair in range(cfg.PAIRS):
            for s in range(cfg.NC):
                t_ccb = ctxf.tile([128, RSL], BF16, name=f"ccb_{pair}_{s}",
                                  tag=f"ccb_{pair}_{s}")
                nc.sync.dma_start(out=t_ccb, in_=a2a_out[pair][s, :, :])
                ccb[(pair, s)] = t_ccb

        # ---- P3: Wo + residual + LayerNorm ---------------------------------
        # row-tile t covers my rows [t*128,(t+1)*128): batch b = t//2,
        # in-slot column range (t%2)*128. Sender 4b+s holds head chunk
        # (pair, s) for that batch. Round A accumulates every pair-0 chunk
        # for ALL row-tiles into SBUF partials while the pair-1 collective
        # is still in flight; round B adds the pair-1 chunks.
        def wo_round(t, pair, pso):
            b = t // 2
            col = slice((t % 2) * 128, (t % 2) * 128 + 128)
            for s in range(cfg.G):
                cc = ccb[(pair, 4 * b + s)][:, col]
                # global output dim chunk for (sender s, pair):
                oc = s * cfg.D4C + pair
                for nsl in range(cfg.WON):
                    nc.tensor.matmul(
                        pso[nsl], cc,
                        wo_sb[:, oc, nsl * cfg.WONW:(nsl + 1) * cfg.WONW],
                        start=s == 0, stop=s == cfg.G - 1)

        partA = []
        for t in range(RQ // 128):
            pso = [psum.tile([128, cfg.WONW], F32, tag="sc",
                             bufs=3, name=f"psoA{nsl}")
                   for nsl in range(cfg.WON)]
            wo_round(t, 0, pso)
            pa = lnp.tile([128, D], BF16, tag="partA", bufs=4)
            for nsl in range(cfg.WON):
                sl = slice(nsl * cfg.WONW, (nsl + 1) * cfg.WONW)
                nc.vector.tensor_add(pa[:, sl], pso[nsl], res_sb[:, t, sl])
            partA.append(pa)

        for t in range(RQ // 128):
            pso = [psum.tile([128, cfg.WONW], F32, tag="sc",
                             bufs=3, name=f"psoB{nsl}")
                   for nsl in range(cfg.WON)]
            wo_round(t, 1, pso)
            x = lnp.tile([128, D], F32, tag="x")
            for nsl in range(cfg.WON):
                sl = slice(nsl * cfg.WONW, (nsl + 1) * cfg.WONW)
                nc.vector.tensor_add(x[:, sl], pso[nsl], partA[t][:, sl])
            fmax = math.gcd(nc.vector.BN_STATS_FMAX, D)
            nsub = D // fmax
            stats = lnp.tile([128, nsub, nc.vector.BN_STATS_DIM], F32,
                             tag="stats")
            for sg in range(nsub):
                nc.vector.bn_stats(
                    out=stats[:, sg, :],
                    in_=x.rearrange("p (a b) -> p a b", a=nsub)[:, sg, :])
            mv = lnp.tile([128, nc.vector.BN_AGGR_DIM], F32, tag="mv")
            nc.vector.bn_aggr(out=mv, in_=stats)
            sd = lnp.tile([128, 1], F32, tag="sd")
            nc.scalar.activation(out=sd, in_=mv[:, 1:2],
                                 func=mybir.ActivationFunctionType.Sqrt,
                                 bias=eps_sb, scale=1.0)
            rstd = lnp.tile([128, 1], F32, tag="rstd")
            nc.vector.reciprocal_approx_fast(rstd, sd)
            y = lnp.tile([128, D], BF16, tag="y")
            nc.vector.tensor_scalar(
                out=y, in0=x, scalar1=mv[:, 0:1], scalar2=rstd,
                op0=mybir.AluOpType.subtract, op1=mybir.AluOpType.mult)
            yg = lnp.tile([128, D], BF16, tag="yg")
            nc.vector.tensor_mul(yg, y, gamma_bc)
            out_sb = lnp.tile([128, D], F32, tag="out_sb")
            nc.vector.tensor_add(out_sb, yg, beta_bc)
            nc.sync.dma_start(out=out_shard[t * 128:(t + 1) * 128, :],
                              in_=out_sb)

    nc.compile()
    return nc


def _tile_x(xT, ns_count, nsw, dc=8):
    """[D, S'] -> [ns, 128, dc, nsw] so each n-slice DMA is contiguous."""
    d, s = xT.shape
    cols = ns_count * nsw
    out = xT[:, :cols].reshape(dc, 128, ns_count, nsw)
    return np.ascontiguousarray(out.transpose(2, 1, 0, 3))


def _tile_w(wT):
    """[D, O] -> [128, dc, O] so the weight DMA is contiguous."""
    d, o = wT.shape
    return np.ascontiguousarray(wT.reshape(d // 128, 128, o).transpose(1, 0, 2))


def make_in_maps(cfg: Cfg, q, k, v, Wq, Wk, Wv, Wo, gamma, beta, sen_len):
    """Host-side sharding: slice/transpose/cast/tile per core."""
    bf = ml_dtypes.bfloat16
    in_maps = []
    woT_full = _tile_w(Wo.T.astype(bf))
    pos = np.arange(cfg.S)
    per_batch = {}
    for b in range(cfg.B):
        per_batch[b] = (
            _tile_x(q[b].T.astype(bf), cfg.NS, cfg.NSW),
            _tile_x(k[b].T.astype(bf), cfg.NS_K, cfg.NSW),
            _tile_x(v[b].T.astype(bf), cfg.KB_MAX, 128),
            np.ascontiguousarray(
                np.where(pos < int(sen_len[b]), 0.0, NEG_INF)
                .astype(np.float32).reshape(cfg.KCH, 128).T),
        )
    for c in range(cfg.NC):
        g = c // cfg.G
        l = c % cfg.G
        hs = slice(l * cfg.D4, (l + 1) * cfg.D4)
        rows = slice(c * cfg.RSL, (c + 1) * cfg.RSL)
        qTb, kTb, vTb, pb = per_batch[g]
        res = np.concatenate([q[b, rows, :] for b in range(cfg.B)], axis=0)
        res = res.astype(np.float32).reshape(cfg.G, 128, cfg.D)
        in_maps.append({
            "qT": qTb, "kT": kTb, "vT": vTb,
            "wqT": _tile_w(Wq[hs, :].T.astype(bf)),
            "wkT": _tile_w(Wk[hs, :].T.astype(bf)),
            "wvT": _tile_w(Wv[hs, :].T.astype(bf)),
            "woT": woT_full,
            "resid": np.ascontiguousarray(res.transpose(1, 0, 2)),
            "pad_bias": pb,
            "gamma": gamma.reshape(1, cfg.D).astype(bf),
            "beta": beta.reshape(1, cfg.D).astype(np.float32),
        })
    return in_maps


def assemble_output(cfg: Cfg, results):
    out = np.empty((cfg.B, cfg.S, cfg.D), np.float32)
    for c in range(cfg.NC):
        rows = slice(c * cfg.RSL, (c + 1) * cfg.RSL)
        for b in range(cfg.B):
            out[b, rows, :] = results[c]["out_shard"][
                b * cfg.RSL:(b + 1) * cfg.RSL]
    return out


_PROGRAM_CACHE = {}


def _get_program(cfg: Cfg):
    key = (cfg.B, cfg.S, cfg.D, cfg.H, cfg.dh, cfg.KB_MAX)
    if key not in _PROGRAM_CACHE:
        _PROGRAM_CACHE[key] = build_program(cfg)
    return _PROGRAM_CACHE[key]


def run(cfg: Cfg, inputs: dict, trace: bool = False):
    nc = _get_program(cfg)
    in_maps = make_in_maps(cfg, **inputs)
    res = run_bass_kernel_spmd(nc, in_maps, core_ids=list(range(cfg.NC)),
                               trace=trace)
    return assemble_output(cfg, res.results), res


def kernel(**inputs) -> np.ndarray:
    kmax = int(np.max(inputs["sen_len"]))
    cfg = Cfg(B=2, S=2048, D=1024, H=16, dh=64, kmax=kmax)
    out, _ = run(cfg, inputs)
    return out
